# revision 1
# baseline (speedup 1.0000x reference)
"""Trainium2 Bass kernel for nn_MHABlock (dense transformer block).

Sharding: data-parallel over batch — 8 cores x 4 batches (2048 tokens/core).
BatchNorm stats are exact via two tiny cross-core AllReduces ([128,2] each).

On-device layout is E-major ("hT" = [E(128 partitions), tokens(free)]) so
BatchNorm / bias / affine ops are per-partition scalars. Attention uses the
"scoresT" formulation (scores transposed: [k_tok, q_tok]): softmax sums
arrive free from an all-ones column prepended to V (row 32*hh of the attnV
PSUM = sum_k exp), no transpose of the attention matrix is needed, and the
exp output feeds attnV directly as the streaming operand. Per-head softmax
denominators land on partitions {0,32,64,96}; reciprocal_approx_fast over
the whole bank + K=1 outer-product matmuls with a ones vector broadcast
1/sum across each head's 32-partition group, so normalization is a single
[128,512] tensor_tensor multiply. Zero rows in the out-projection weights
kill all pad rows. q/k/exp/V run in bf16 (PE fp32 is half-rate).
"""

import numpy as np

B, N, D_IN, E, H, KD, FF = 32, 512, 2, 128, 8, 16, 512
NCORES = 8
BPC = B // NCORES          # batches per core
T = BPC * N                # 2048 local tokens
NTOK = B * N               # global token count for BN
NORM = 1.0 / np.sqrt(16.0)
EPS = 1e-5

_CACHE = {}
LAST_RESULT = None


def _build_nc():
    import concourse.bass as bass  # noqa: F401
    import concourse.mybir as mybir
    import concourse.tile as tile
    from concourse import bacc

    f32 = mybir.dt.float32
    bf16 = mybir.dt.bfloat16
    Act = mybir.ActivationFunctionType
    Alu = mybir.AluOpType
    AX = mybir.AxisListType

    nc = bacc.Bacc("TRN2", target_bir_lowering=False, debug=False,
                   enable_asserts=False, num_devices=NCORES)

    # ---- DRAM I/O ----
    d_xT = nc.dram_tensor("xT", [D_IN, T], f32, kind="ExternalInput").ap()
    d_We1 = nc.dram_tensor("We1", [D_IN, E], f32, kind="ExternalInput").ap()
    d_WqQ = nc.dram_tensor("WqQ", [E, 256], bf16, kind="ExternalInput").ap()
    d_WkQ = nc.dram_tensor("WkQ", [E, 256], bf16, kind="ExternalInput").ap()
    d_WvI = nc.dram_tensor("WvI", [E, 128], bf16, kind="ExternalInput").ap()
    d_WoQ = nc.dram_tensor("WoQ", [128, 256], bf16, kind="ExternalInput").ap()
    d_fW1 = nc.dram_tensor("fW1", [E, FF], bf16, kind="ExternalInput").ap()
    d_fW2 = nc.dram_tensor("fW2", [128, 512], bf16, kind="ExternalInput").ap()
    d_vecs = nc.dram_tensor("vecs", [128, 12], f32, kind="ExternalInput").ap()
    d_yT = nc.dram_tensor("yT", [E, T], f32, kind="ExternalOutput").ap()

    RG = [list(range(NCORES))]

    with tile.TileContext(nc) as tc:
        with tc.sbuf_pool(name="sb", bufs=1) as sb, \
             tc.psum_pool(name="ps", bufs=1) as ps, \
             tc.tile_pool(name="dr", bufs=1, space="DRAM") as dr:

            def P(shape, dt, name):  # persistent tile
                return sb.tile(shape, dt, name=name, tag=name, bufs=1)

            xT = P([D_IN, T], f32, "xT_sb")
            We1_sb = P([D_IN, E], f32, "We1_sb")
            WqQ_sb = P([128, 256], bf16, "WqQ_sb")
            WkQ_sb = P([128, 256], bf16, "WkQ_sb")
            WvI_sb = P([128, 128], bf16, "WvI_sb")
            WoQ_sb = P([128, 256], bf16, "WoQ_sb")
            fW1_sb = P([128, FF], bf16, "fW1_sb")
            fW2_sb = P([128, 512], bf16, "fW2_sb")
            vecs_sb = P([128, 12], f32, "vecs_sb")
            ones_sb = P([128, 32], f32, "ones_sb")

            H0T = P([128, T], f32, "H0T")
            H0b = P([128, T], bf16, "H0b")
            qT = [P([128, T], bf16, f"qT{g}") for g in range(2)]
            kT = [P([128, T], bf16, f"kT{g}") for g in range(2)]
            V_aug = P([128, 16 * 256], bf16, "V_aug")
            HT = [P([128, T], bf16, f"HT{g}") for g in range(2)]
            h1T = P([128, T], f32, "h1T")
            h1nT = P([128, T], f32, "h1nT")
            h1nb = P([128, T], bf16, "h1nb")
            h2T = [P([128, T], bf16, f"h2T{qf}") for qf in range(4)]
            yT = P([128, T], f32, "yT_sb")
            sq = P([128, T], f32, "sq")
            st1 = P([128, 2], f32, "st1")
            st2 = P([128, 2], f32, "st2")
            gst1 = P([128, 2], f32, "gst1")
            gst2 = P([128, 2], f32, "gst2")
            bn1s = P([128, 6], f32, "bn1s")
            bn2s = P([128, 6], f32, "bn2s")

            # ---- load inputs ----
            nc.sync.dma_start(xT[:], d_xT)
            nc.sync.dma_start(We1_sb[:], d_We1)
            nc.sync.dma_start(WqQ_sb[:], d_WqQ)
            nc.sync.dma_start(WkQ_sb[:], d_WkQ)
            nc.sync.dma_start(WvI_sb[:], d_WvI)
            nc.sync.dma_start(WoQ_sb[:], d_WoQ)
            nc.sync.dma_start(fW1_sb[:], d_fW1)
            nc.sync.dma_start(fW2_sb[:], d_fW2)
            nc.sync.dma_start(vecs_sb[:], d_vecs)
            nc.vector.memset(ones_sb[:], 1.0)
            nc.vector.memset(V_aug[:], 0.0)
            va_ones = V_aug.rearrange("p (t h w) -> p (t h) w", t=16, h=8)[:, :, 0:1]
            nc.gpsimd.memset(va_ones, 1.0)

            # ---- Phase A: embedding h0 = x @ We1 + be1 (E-major) ----
            for c in range(4):
                pm = ps.tile([128, 512], f32, tag="mm", bufs=2, name=f"pm_e{c}")
                nc.tensor.matmul(pm[:], lhsT=We1_sb[:],
                                 rhs=xT[:, 512 * c:512 * (c + 1)],
                                 start=True, stop=True)
                nc.vector.tensor_scalar_add(H0T[:, 512 * c:512 * (c + 1)],
                                            pm[:], vecs_sb[:, 0:1])

            nc.vector.tensor_copy(H0b[:], H0T[:])

            # ---- Phase B: q/k projections (quad-padded, bf16 out) ----
            for g in range(2):
                for c in range(4):
                    pq = ps.tile([128, 512], f32, tag="mm", bufs=2,
                                 name=f"pq{g}{c}")
                    nc.tensor.matmul(pq[:], lhsT=WqQ_sb[:, 128 * g:128 * (g + 1)],
                                     rhs=H0b[:, 512 * c:512 * (c + 1)],
                                     start=True, stop=True)
                    nc.vector.tensor_copy(qT[g][:, 512 * c:512 * (c + 1)], pq[:])
                    pk = ps.tile([128, 512], f32, tag="mm", bufs=2,
                                 name=f"pk{g}{c}")
                    nc.tensor.matmul(pk[:], lhsT=WkQ_sb[:, 128 * g:128 * (g + 1)],
                                     rhs=H0b[:, 512 * c:512 * (c + 1)],
                                     start=True, stop=True)
                    nc.vector.tensor_copy(kT[g][:, 512 * c:512 * (c + 1)], pk[:])

            # ---- v projection into V_aug (token-major, 32-blocks +ones) ----
            for t in range(16):
                pv = ps.tile([128, 128], f32, tag="mm", bufs=2, name=f"pv{t}")
                nc.tensor.matmul(pv[:], lhsT=H0b[:, 128 * t:128 * (t + 1)],
                                 rhs=WvI_sb[:], start=True, stop=True)
                dst = V_aug[:, 256 * t:256 * (t + 1)]
                dst = dst.rearrange("p (h w) -> p h w", h=8)[:, :, 1:17]
                src = pv.rearrange("p (h w) -> p h w", h=8)
                nc.vector.tensor_copy(dst, src)

            # ---- Phase C: attention ----
            for b in range(4):
                for g in range(2):
                    av = ps.tile([128, 512], f32, tag="av", bufs=2,
                                 name=f"av{b}{g}")
                    for c in range(4):
                        for hp in range(2):
                            scp = ps.tile([128, 1024], f32, tag="sc", bufs=2,
                                          name=f"scp{b}{g}{c}{hp}")
                            for j in range(2):
                                hh = 2 * hp + j
                                nc.tensor.matmul(
                                    scp[:, 512 * j:512 * (j + 1)],
                                    lhsT=kT[g][32 * hh:32 * (hh + 1),
                                               512 * b + 128 * c:
                                               512 * b + 128 * (c + 1)],
                                    rhs=qT[g][32 * hh:32 * (hh + 1),
                                              512 * b:512 * (b + 1)],
                                    start=True, stop=True,
                                    tile_position=(32 * hh, 0))
                            ex = sb.tile([128, 1024], bf16, tag="ex", bufs=3,
                                         name=f"ex{b}{g}{c}{hp}")
                            nc.scalar.activation(ex[:], scp[:], Act.Exp,
                                                 scale=float(NORM))
                            for j in range(2):
                                hh = 2 * hp + j
                                h = 4 * g + hh
                                tci = 4 * b + c
                                nc.tensor.matmul(
                                    av[32 * hh:32 * (hh + 1), :],
                                    lhsT=V_aug[:, 256 * tci + 32 * h:
                                               256 * tci + 32 * h + 32],
                                    rhs=ex[:, 512 * j:512 * (j + 1)],
                                    start=(c == 0), stop=(c == 3),
                                    tile_position=(0, 32 * hh))
                    # normalize: raw * (1/sums), broadcast via K=1 matmul
                    raw = sb.tile([128, 512], f32, tag="raw", bufs=2,
                                  name=f"raw{b}{g}")
                    nc.vector.tensor_copy(raw[:], av[:])
                    rec = sb.tile([128, 512], f32, tag="rec", bufs=2,
                                  name=f"rec{b}{g}")
                    nc.vector.reciprocal_approx_fast(rec[:], av[:])
                    Rp = ps.tile([128, 512], f32, tag="mm", bufs=2,
                                 name=f"Rp{b}{g}")
                    for hh in range(4):
                        nc.tensor.matmul(
                            Rp[32 * hh:32 * (hh + 1), :],
                            lhsT=ones_sb[32 * hh:32 * hh + 1, :],
                            rhs=rec[32 * hh:32 * hh + 1, :],
                            start=True, stop=True,
                            tile_position=(32 * hh, 32 * hh))
                    nc.vector.tensor_mul(HT[g][:, 512 * b:512 * (b + 1)],
                                         raw[:], Rp[:])

                # out-projection + skip (both quads accumulate in PSUM)
                po = ps.tile([128, 512], f32, tag="mm", bufs=2, name=f"po{b}")
                for g in range(2):
                    nc.tensor.matmul(po[:], lhsT=WoQ_sb[:, 128 * g:128 * (g + 1)],
                                     rhs=HT[g][:, 512 * b:512 * (b + 1)],
                                     start=(g == 0), stop=(g == 1))
                nc.vector.tensor_add(h1T[:, 512 * b:512 * (b + 1)], po[:],
                                     H0T[:, 512 * b:512 * (b + 1)])

            # ---- BatchNorm helper (exact, cross-core stats) ----
            def batchnorm(src, st, gst, bns, wcol, bcol, ccname):
                nc.vector.reduce_sum(out=st[:, 0:1], in_=src[:], axis=AX.X)
                nc.scalar.activation(sq[:], src[:], Act.Square,
                                     accum_out=st[:, 1:2])
                cc_in = dr.tile([128, 2], f32, name=f"{ccname}_in",
                                tag=f"{ccname}_in")
                cc_out = dr.tile([128, 2], f32, addr_space="Shared",
                                 name=f"{ccname}_out", tag=f"{ccname}_out")
                nc.sync.dma_start(cc_in[:], st[:])
                nc.gpsimd.collective_compute(
                    "AllReduce", Alu.add, replica_groups=RG,
                    ins=[cc_in[:]], outs=[cc_out[:]])
                nc.sync.dma_start(gst[:], cc_out[:])
                inv_n = 1.0 / float(NTOK)
                nc.vector.tensor_scalar_mul(bns[:, 0:1], gst[:, 0:1], inv_n)
                nc.vector.tensor_scalar_mul(bns[:, 1:2], gst[:, 1:2], inv_n)
                nc.vector.tensor_mul(bns[:, 4:5], bns[:, 0:1], bns[:, 0:1])
                nc.vector.tensor_sub(bns[:, 1:2], bns[:, 1:2], bns[:, 4:5])
                nc.scalar.activation(bns[:, 5:6], bns[:, 1:2], Act.Sqrt,
                                     bias=vecs_sb[:, 9:10])
                nc.vector.reciprocal(bns[:, 2:3], bns[:, 5:6])
                nc.vector.tensor_mul(bns[:, 2:3], bns[:, 2:3],
                                     vecs_sb[:, wcol:wcol + 1])
                nc.vector.tensor_mul(bns[:, 4:5], bns[:, 0:1], bns[:, 2:3])
                nc.vector.tensor_sub(bns[:, 3:4], vecs_sb[:, bcol:bcol + 1],
                                     bns[:, 4:5])

            # ---- BN1 ----
            batchnorm(h1T, st1, gst1, bn1s, 1, 2, "cc1")
            for c in range(4):
                nc.vector.tensor_scalar(
                    h1nT[:, 512 * c:512 * (c + 1)],
                    h1T[:, 512 * c:512 * (c + 1)],
                    bn1s[:, 2:3], bn1s[:, 3:4], op0=Alu.mult, op1=Alu.add)
                nc.vector.tensor_copy(h1nb[:, 512 * c:512 * (c + 1)],
                                      h1nT[:, 512 * c:512 * (c + 1)])

            # ---- FFN (ffb2 cancels inside BN2) ----
            for qf in range(4):
                for c in range(4):
                    pf = ps.tile([128, 512], f32, tag="mm", bufs=2,
                                 name=f"pf{qf}{c}")
                    nc.tensor.matmul(pf[:],
                                     lhsT=fW1_sb[:, 128 * qf:128 * (qf + 1)],
                                     rhs=h1nb[:, 512 * c:512 * (c + 1)],
                                     start=True, stop=True)
                    nc.vector.tensor_scalar(
                        h2T[qf][:, 512 * c:512 * (c + 1)], pf[:],
                        vecs_sb[:, 3 + qf:4 + qf], 0.0,
                        op0=Alu.add, op1=Alu.max)
            for c in range(4):
                p2 = ps.tile([128, 512], f32, tag="mm", bufs=2, name=f"p2{c}")
                for qf in range(4):
                    nc.tensor.matmul(p2[:],
                                     lhsT=fW2_sb[:, 128 * qf:128 * (qf + 1)],
                                     rhs=h2T[qf][:, 512 * c:512 * (c + 1)],
                                     start=(qf == 0), stop=(qf == 3))
                nc.vector.tensor_add(yT[:, 512 * c:512 * (c + 1)], p2[:],
                                     h1nT[:, 512 * c:512 * (c + 1)])

            # ---- BN2 + output ----
            batchnorm(yT, st2, gst2, bn2s, 7, 8, "cc2")
            for c in range(4):
                nc.vector.tensor_scalar(
                    sq[:, 512 * c:512 * (c + 1)], yT[:, 512 * c:512 * (c + 1)],
                    bn2s[:, 2:3], bn2s[:, 3:4], op0=Alu.mult, op1=Alu.add)
                nc.sync.dma_start(d_yT[:, 512 * c:512 * (c + 1)],
                                  sq[:, 512 * c:512 * (c + 1)])

    nc.compile()
    return nc


def _host_prep(inputs):
    f = np.float32
    Wq, Wk, Wv, Wo = (np.asarray(inputs[k], f) for k in ("Wq", "Wk", "Wv", "Wo"))
    WqQ = np.zeros((2, E, 128), f)
    WkQ = np.zeros((2, E, 128), f)
    WoQ = np.zeros((2, 128, E), f)
    for g in range(2):
        for hh in range(4):
            h = 4 * g + hh
            WqQ[g, :, 32 * hh:32 * hh + 16] = Wq[h]
            WkQ[g, :, 32 * hh:32 * hh + 16] = Wk[h]
            WoQ[g, 32 * hh + 1:32 * hh + 17, :] = Wo[h]
    WvI = np.ascontiguousarray(np.transpose(Wv, (1, 0, 2)).reshape(E, H * KD))
    fW2 = np.ascontiguousarray(
        np.asarray(inputs["ffW2"], f).reshape(4, 128, E).transpose(1, 0, 2))
    vecs = np.zeros((128, 12), f)
    vecs[:, 0] = inputs["be1"]
    vecs[:, 1] = inputs["bn1_w"]
    vecs[:, 2] = inputs["bn1_b"]
    vecs[:, 3:7] = np.asarray(inputs["ffb1"], f).reshape(4, 128).T
    vecs[:, 7] = inputs["bn2_w"]
    vecs[:, 8] = inputs["bn2_b"]
    vecs[:, 9] = EPS
    import ml_dtypes
    bf = ml_dtypes.bfloat16
    return {
        "We1": np.ascontiguousarray(np.asarray(inputs["We1"], f)),
        "WqQ": np.ascontiguousarray(np.concatenate([WqQ[0], WqQ[1]], axis=1)).astype(bf),
        "WkQ": np.ascontiguousarray(np.concatenate([WkQ[0], WkQ[1]], axis=1)).astype(bf),
        "WvI": WvI.astype(bf),
        "WoQ": np.ascontiguousarray(np.concatenate([WoQ[0], WoQ[1]], axis=1)).astype(bf),
        "fW1": np.ascontiguousarray(np.asarray(inputs["ffW1"], f)).astype(bf),
        "fW2": np.ascontiguousarray(fW2.reshape(128, 512)).astype(bf), "vecs": vecs,
    }


def _get_runner():
    """Build the sharded jitted executable once and cache it."""
    if "runner" in _CACHE:
        return _CACHE["runner"]
    import jax
    import concourse.mybir as mybir
    from jax.sharding import Mesh, PartitionSpec
    from jax.experimental.shard_map import shard_map
    from concourse.bass2jax import (_bass_exec_p, install_neuronx_cc_hook,
                                    partition_id_tensor)

    if "nc" not in _CACHE:
        _CACHE["nc"] = _build_nc()
    nc = _CACHE["nc"]
    install_neuronx_cc_hook()
    assert nc.dbg_addr is None

    partition_name = (nc.partition_id_tensor.name
                      if nc.partition_id_tensor else None)
    in_names, out_names, out_avals, zero_outs = [], [], [], []
    for alloc in nc.m.functions[0].allocations:
        if not isinstance(alloc, mybir.MemoryLocationSet):
            continue
        name = alloc.memorylocations[0].name
        if alloc.kind == "ExternalInput":
            if name != partition_name:
                in_names.append(name)
        elif alloc.kind == "ExternalOutput":
            shape = tuple(alloc.tensor_shape)
            dtype = mybir.dt.np(alloc.dtype)
            out_names.append(name)
            out_avals.append(jax.core.ShapedArray(shape, dtype))
            zero_outs.append(np.zeros(shape, dtype))
    n_params = len(in_names)
    n_outs = len(out_avals)
    all_in_names = list(in_names) + list(out_names)
    if partition_name is not None:
        all_in_names.append(partition_name)
    donate = tuple(range(n_params, n_params + n_outs))

    def _body(*args):
        operands = list(args)
        if partition_name is not None:
            operands.append(partition_id_tensor())
        outs = _bass_exec_p.bind(
            *operands,
            out_avals=tuple(out_avals),
            in_names=tuple(all_in_names),
            out_names=tuple(out_names),
            lowering_input_output_aliases=(),
            sim_require_finite=True,
            sim_require_nnan=True,
            nc=nc,
        )
        return tuple(outs)

    devices = jax.devices()[:NCORES]
    mesh = Mesh(np.asarray(devices), ("core",))
    in_specs = (PartitionSpec("core"),) * (n_params + n_outs)
    out_specs = (PartitionSpec("core"),) * len(out_names)
    sharded = jax.jit(
        shard_map(_body, mesh=mesh, in_specs=in_specs, out_specs=out_specs,
                  check_rep=False),
        donate_argnums=donate, keep_unused=True)

    def run(in_maps):
        per_core = [[np.asarray(m[name]) for name in in_names]
                    for m in in_maps]
        concat_in = [np.concatenate([per_core[c][i] for c in range(NCORES)],
                                    axis=0) for i in range(n_params)]
        concat_zeros = [np.zeros((NCORES * z.shape[0], *z.shape[1:]), z.dtype)
                        for z in zero_outs]
        out_arrs = sharded(*concat_in, *concat_zeros)
        out_arrs = [np.asarray(a) for a in out_arrs]
        return [{name: out_arrs[i].reshape(NCORES, *out_avals[i].shape)[c]
                 for i, name in enumerate(out_names)}
                for c in range(NCORES)]

    _CACHE["runner"] = run
    return run


def _make_in_maps(inputs):
    shared = _host_prep(inputs)
    x1 = np.asarray(inputs["x1"], np.float32)
    in_maps = []
    for cidx in range(NCORES):
        m = dict(shared)
        xl = x1[BPC * cidx:BPC * (cidx + 1)].reshape(T, D_IN)
        m["xT"] = np.ascontiguousarray(xl.T)
        in_maps.append(m)
    return in_maps


def kernel(**inputs):
    run = _get_runner()
    results = run(_make_in_maps(inputs))
    outs = []
    for cidx in range(NCORES):
        yTo = results[cidx]["yT"]          # [E, T]
        outs.append(np.ascontiguousarray(yTo.T).reshape(BPC, N, E))
    return np.concatenate(outs, 0).astype(np.float32)



# revision 14
# speedup vs baseline: 1.5601x; 1.5601x over previous
"""Trainium2 Bass kernel for nn_MHABlock (dense transformer block).

Sharding: data-parallel over batch - 8 cores x 4 batches (2048 tokens/core).
BatchNorm stats are exact via two AllGather collectives of per-core
(mean, var) pairs ([128,2] each) combined locally on every core.

Layout: E-major ("T" = [E(128 partitions), tokens(free)]).  Attention uses
the scoresT formulation (scores [k_tok, q_tok]) for exp, but attnV is
*flipped*: out[q_tok, v] with N=17 per matmul (16 v dims + a ones column of
V giving the softmax denominator per q partition).  Normalization is then a
per-partition reciprocal + one broadcast multiply on DVE; heads are
transposed back to (h,v)-major with PE transpose-mode and fed to the output
projection.  Softmax exp is split between the ACT engine (table exp) and the
DVE (Schraudolph bf16-bit exp via f32->int16 convert).  PSUM->SBUF moves
that need no dtype change (H0, q/k in f32->f32r, transposed heads) go over
DMA engines instead of DVE/ACT.  FFN consumes unnormalized h1 with
BN1-scaled weights (a*W1) so the BN1 apply is off the critical path.
"""

import numpy as np

B, N, D_IN, E, H, KD, FF = 32, 512, 2, 128, 8, 16, 512
NCORES = 8
BPC = B // NCORES          # batches per core
T = BPC * N                # 2048 local tokens
NTOK = B * N               # global token count for BN
NORM = 1.0 / np.sqrt(16.0)
EPS = 1e-5

# Schraudolph bf16-bit exp:  i16 = trunc(scores * SCH_A + SCH_B),
# bits reinterpreted as bf16 ~= exp(NORM * scores).  Calibrated for the
# truncating f32->int16 convert (C=6.9).
SCH_A = float(0.25 * 128 * np.log2(np.e))
SCH_B = 16256.0 - 6.9
# per-batch exp tile indices (of 16) computed on DVE instead of ACT
DVE_EXP = frozenset({2, 5, 8, 11, 13})

_CACHE = {}
LAST_RESULT = None


def _build_nc():
    import concourse.bass as bass  # noqa: F401
    import concourse.mybir as mybir
    import concourse.tile as tile
    from concourse import bacc

    f32 = mybir.dt.float32
    f32r = mybir.dt.float32r
    bf16 = mybir.dt.bfloat16
    i16 = mybir.dt.int16
    Act = mybir.ActivationFunctionType
    Alu = mybir.AluOpType
    AX = mybir.AxisListType

    nc = bacc.Bacc("TRN2", target_bir_lowering=False, debug=False,
                   enable_asserts=False, num_devices=NCORES)

    # ---- DRAM I/O ----
    d_xTa = nc.dram_tensor("xTa", [3, T], f32r, kind="ExternalInput").ap()
    d_We1a = nc.dram_tensor("We1a", [3, E], f32r, kind="ExternalInput").ap()
    d_WqQ = nc.dram_tensor("WqQ", [E, 256], f32r, kind="ExternalInput").ap()
    d_WkQ = nc.dram_tensor("WkQ", [E, 256], f32r, kind="ExternalInput").ap()
    d_WvA = nc.dram_tensor("WvA", [E, 128], bf16, kind="ExternalInput").ap()
    d_WoA = nc.dram_tensor("WoA", [128, E], bf16, kind="ExternalInput").ap()
    d_fW1f = nc.dram_tensor("fW1f", [E, FF], f32, kind="ExternalInput").ap()
    d_fW1b = nc.dram_tensor("fW1b", [E, FF], bf16, kind="ExternalInput").ap()
    d_fW2q = nc.dram_tensor("fW2q", [128, 512], bf16, kind="ExternalInput").ap()
    d_ident = nc.dram_tensor("ident", [128, 128], bf16, kind="ExternalInput").ap()
    d_vecs = nc.dram_tensor("vecs", [128, 12], f32, kind="ExternalInput").ap()
    d_yT = nc.dram_tensor("yT", [E, T], f32, kind="ExternalOutput").ap()

    RG = [list(range(NCORES))]

    with tile.TileContext(nc) as tc:
        with tc.sbuf_pool(name="sb", bufs=1) as sb, \
             tc.psum_pool(name="ps", bufs=1) as ps, \
             tc.tile_pool(name="dr", bufs=1, space="DRAM") as dr:

            def P(shape, dt, name):  # persistent tile
                return sb.tile(shape, dt, name=name, tag=name, bufs=1)

            xTa = P([3, T], f32r, "xTa_sb")
            We1a_sb = P([3, E], f32r, "We1a_sb")
            WqQ_sb = P([128, 256], f32r, "WqQ_sb")
            WkQ_sb = P([128, 256], f32r, "WkQ_sb")
            WvA_sb = P([128, 128], bf16, "WvA_sb")
            WoA_sb = P([128, 128], bf16, "WoA_sb")
            fW1f_sb = P([128, FF], f32, "fW1f_sb")
            fW1b_sb = P([128, FF], bf16, "fW1b_sb")
            fW1s_sb = P([128, FF], f32r, "fW1s_sb")
            fW2q_sb = P([128, 512], bf16, "fW2q_sb")
            ident_sb = P([128, 128], bf16, "ident_sb")
            vecs_sb = P([128, 12], f32, "vecs_sb")

            H0T = P([128, T], f32r, "H0T")
            H0b = P([128, T], bf16, "H0b")
            qT = [P([128, T], f32r, f"qT{g}") for g in range(2)]
            kT = [P([128, T], f32r, f"kT{g}") for g in range(2)]
            V17 = P([128, 16 * 136], bf16, "V17")
            h1T = P([128, T], f32r, "h1T")
            h1nT = P([128, T], f32, "h1nT")
            yT = P([128, T], f32, "yT_sb")
            sqo = P([128, T], f32, "sqo")
            h2T = [P([128, T], bf16, f"h2T{qf}") for qf in range(4)]
            hTs = [P([128, 512], bf16, f"hTs{b}") for b in range(4)]
            sb1 = P([128, 24], f32, "sb1")
            sb2 = P([128, 24], f32, "sb2")
            mv1 = P([128, 2], f32, "mv1")
            mv2 = P([128, 2], f32, "mv2")
            G1 = P([128, 16], f32, "G1")
            G2 = P([128, 16], f32, "G2")
            cmb = P([128, 24], f32, "cmb")
            biasq = P([128, 4], f32, "biasq")
            b1pb = P([128, 1], bf16, "b1pb")

            # ---- load inputs (xTa/We1a first: they gate the embedding) ----
            nc.sync.dma_start(xTa[:], d_xTa)
            nc.sync.dma_start(We1a_sb[:], d_We1a)
            nc.sync.dma_start(WqQ_sb[:], d_WqQ)
            nc.sync.dma_start(WkQ_sb[:], d_WkQ)
            nc.sync.dma_start(WvA_sb[:], d_WvA)
            nc.sync.dma_start(WoA_sb[:], d_WoA)
            nc.sync.dma_start(ident_sb[:], d_ident)
            nc.sync.dma_start(vecs_sb[:], d_vecs)
            nc.sync.dma_start(fW2q_sb[:], d_fW2q)
            nc.sync.dma_start(fW1f_sb[:], d_fW1f)
            nc.sync.dma_start(fW1b_sb[:], d_fW1b)

            # ones columns of V17 (col 16 of every 17-block)
            v17v = V17.rearrange("p (t h s) -> p t h s", t=16, h=8)
            nc.gpsimd.memset(v17v[:, :, :, 16:17], 1.0)

            # ---- Phase A: embedding  h0 = [x|1] @ [We1;be1]  (E-major) ----
            for cp in range(2):
                pm = ps.tile([128, 1024], f32, tag="sc", bufs=2, name=f"pm{cp}")
                for j in range(2):
                    c = 2 * cp + j
                    nc.tensor.matmul(pm[:, 512 * j:512 * (j + 1)],
                                     lhsT=We1a_sb[:],
                                     rhs=xTa[:, 512 * c:512 * (c + 1)],
                                     start=True, stop=True)
                nc.scalar.copy(H0T[:, 1024 * cp:1024 * (cp + 1)], pm[:])
                nc.gpsimd.tensor_copy(H0b[:, 1024 * cp:1024 * (cp + 1)],
                                      H0T[:, 1024 * cp:1024 * (cp + 1)])

            H0r = H0T

            # ---- Phase B: q/k projections (quad-padded, f32 via DMA) ----
            for g in range(2):
                for cp in range(2):
                    pq = ps.tile([128, 1024], f32, tag="sc", bufs=2,
                                 name=f"pq{g}{cp}")
                    pk = ps.tile([128, 1024], f32, tag="sc", bufs=2,
                                 name=f"pk{g}{cp}")
                    for j in range(2):
                        c = 2 * cp + j
                        nc.tensor.matmul(pq[:, 512 * j:512 * (j + 1)],
                                         lhsT=WqQ_sb[:, 128 * g:128 * (g + 1)],
                                         rhs=H0r[:, 512 * c:512 * (c + 1)],
                                         start=True, stop=True)
                        nc.tensor.matmul(pk[:, 512 * j:512 * (j + 1)],
                                         lhsT=WkQ_sb[:, 128 * g:128 * (g + 1)],
                                         rhs=H0r[:, 512 * c:512 * (c + 1)],
                                         start=True, stop=True)
                    nc.vector.tensor_copy(qT[g][:, 1024 * cp:1024 * (cp + 1)],
                                          pq[:])
                    nc.scalar.copy(kT[g][:, 1024 * cp:1024 * (cp + 1)], pk[:])

            # ---- Phase C: v projection into V17 (token-major, +ones col) ----
            for t in range(16):
                pv = ps.tile([128, 128], f32, tag="tp", bufs=1, name=f"pv{t}")
                nc.tensor.matmul(pv[:], lhsT=H0b[:, 128 * t:128 * (t + 1)],
                                 rhs=WvA_sb[:], start=True, stop=True)
                dst = v17v[:, t, :, 0:16]
                src = pv.rearrange("p (h s) -> p h s", h=8)
                if t % 2 == 0:
                    nc.vector.tensor_copy(dst, src)
                else:
                    nc.scalar.copy(dst, src)

            # ---- Phase D: attention, software-pipelined over batches ----
            ex_tiles = {}

            def emit_scores_exp(b):
                for h in range(8):
                    g, hh = h // 4, h % 4
                    for cp in range(2):
                        scp = ps.tile([128, 1024], f32, tag="sc", bufs=2,
                                      name=f"scp{b}{h}{cp}")
                        for j in range(2):
                            c = 2 * cp + j
                            nc.tensor.matmul(
                                scp[:, 512 * j:512 * (j + 1)],
                                lhsT=kT[g][
                                    32 * hh:32 * (hh + 1),
                                    512 * b + 128 * c:512 * b + 128 * (c + 1)],
                                rhs=qT[g][
                                    32 * hh:32 * (hh + 1),
                                    512 * b:512 * (b + 1)],
                                start=True, stop=True,
                                tile_position=(32 * hh, 0))
                        ex = sb.tile([128, 1024], bf16, tag="ex", bufs=8,
                                     name=f"ex{b}{h}{cp}")
                        idx = 2 * h + cp
                        if idx in DVE_EXP:
                            nc.vector.tensor_scalar(
                                ex.bitcast(i16)[:], scp[:], SCH_A, SCH_B,
                                op0=Alu.mult, op1=Alu.add)
                        else:
                            nc.scalar.activation(ex[:], scp[:], Act.Exp,
                                                 scale=float(NORM))
                        ex_tiles[(b, h, cp)] = ex

            def emit_attn_tail(b):
                # attnV: out [q, 17] per (h, qc), accumulated over 4 k-chunks
                av = ps.tile([128, 1024], f32, tag="av", bufs=1,
                             name=f"av{b}")
                for h in range(8):
                    for qc in range(4):
                        for kc in range(4):
                            cp, j = kc // 2, kc % 2
                            ex = ex_tiles[(b, h, cp)]
                            nc.tensor.matmul(
                                av[:, 256 * qc + 17 * h:256 * qc + 17 * h + 17],
                                lhsT=ex[:, 512 * j + 128 * qc:
                                        512 * j + 128 * (qc + 1)],
                                rhs=V17[:, 136 * (4 * b + kc) + 17 * h:
                                        136 * (4 * b + kc) + 17 * (h + 1)],
                                start=(kc == 0), stop=(kc == 3))
                # normalize by the ones-column sums (per q partition)
                av4 = av.rearrange("p (q x) -> p q x", q=4)[:, :, :136]
                av5 = av4.rearrange("p q (h s) -> p q h s", h=8)
                rd = sb.tile([128, 32], f32, tag="rd", bufs=2, name=f"rd{b}")
                rd4 = rd.rearrange("p (q h s) -> p q h s", q=4, h=8)
                nc.vector.reciprocal(rd4, av5[:, :, :, 16:17])
                hn = sb.tile([128, 512], bf16, tag="hn", bufs=2, name=f"hn{b}")
                hn4 = hn.rearrange("p (q h s) -> p q h s", q=4, h=8)
                vals = av5[:, :, :, 0:16]
                rdb = bass.broadcast_tensor_aps(vals, rd4)[1]
                nc.vector.tensor_mul(hn4, vals, rdb)
                # transpose back to (h,v)-major and project
                tp = ps.tile([128, 512], bf16, tag="tp", bufs=1, name=f"tp{b}")
                for qc in range(4):
                    nc.tensor.transpose(tp[:, 128 * qc:128 * (qc + 1)],
                                        hn[:, 128 * qc:128 * (qc + 1)],
                                        ident_sb[:])
                nc.vector.tensor_copy(hTs[b][:], tp[:])
                po = ps.tile([128, 512], f32, tag="po", bufs=1, name=f"po{b}")
                nc.tensor.matmul(po[:], lhsT=WoA_sb[:], rhs=hTs[b][:],
                                 start=True, stop=True)
                nc.vector.tensor_add(h1T[:, 512 * b:512 * (b + 1)], po[:],
                                     H0T[:, 512 * b:512 * (b + 1)])
                nc.vector.bn_stats(sb1[:, 6 * b:6 * (b + 1)],
                                   h1T[:, 512 * b:512 * (b + 1)])

            for b in range(5):
                if b < 4:
                    emit_scores_exp(b)
                if b > 0:
                    emit_attn_tail(b - 1)

            # ---- cross-core BN stats helper ----
            def bn_round(mv, Gt, ccname, wcol, bcol, col0):
                """AllGather per-core (mean,var); combine; produce
                a = w*rstd (cmb col0), bp = b - mean*a (col0+1)."""
                cc_in = dr.tile([128, 2], f32, name=f"{ccname}_in",
                                tag=f"{ccname}_in")
                cc_out = dr.tile([NCORES, 128, 2], f32, addr_space="Shared",
                                 name=f"{ccname}_out", tag=f"{ccname}_out")
                nc.sync.dma_start(cc_in[:], mv[:])
                nc.gpsimd.collective_compute(
                    "AllGather", Alu.bypass, replica_groups=RG,
                    ins=[cc_in[:]], outs=[cc_out[:]])
                nc.sync.dma_start(
                    Gt.rearrange("p (r s) -> p r s", r=8),
                    cc_out.rearrange("r p s -> p r s"))
                G3 = Gt.rearrange("p (r s) -> p r s", r=8)
                means, vars_ = G3[:, :, 0:1], G3[:, :, 1:2]
                c = cmb
                t1 = c[:, col0 + 2:col0 + 10].rearrange("p (r s) -> p r s", r=8)
                nc.vector.tensor_mul(t1, means, means)
                nc.vector.tensor_add(t1, t1, vars_)
                nc.vector.reduce_sum(out=c[:, col0 + 10:col0 + 11],
                                     in_=means[:, :, 0], axis=AX.X)
                nc.vector.reduce_sum(out=c[:, col0 + 11:col0 + 12],
                                     in_=t1[:, :, 0], axis=AX.X)
                mean = c[:, col0 + 4:col0 + 5]
                nc.vector.tensor_scalar_mul(mean, c[:, col0 + 10:col0 + 11],
                                            0.125)
                e2 = c[:, col0 + 5:col0 + 6]
                nc.vector.tensor_scalar_mul(e2, c[:, col0 + 11:col0 + 12],
                                            0.125)
                m2 = c[:, col0 + 6:col0 + 7]
                nc.vector.tensor_mul(m2, mean, mean)
                var = c[:, col0 + 7:col0 + 8]
                nc.vector.tensor_sub(var, e2, m2)
                sd = c[:, col0 + 8:col0 + 9]
                nc.scalar.activation(sd, var, Act.Sqrt, bias=vecs_sb[:, 4:5])
                rstd = c[:, col0 + 9:col0 + 10]
                nc.vector.reciprocal(rstd, sd)
                a = c[:, col0:col0 + 1]
                nc.vector.tensor_mul(a, rstd, vecs_sb[:, wcol:wcol + 1])
                nc.vector.tensor_mul(m2, mean, a)
                bp = c[:, col0 + 1:col0 + 2]
                nc.vector.tensor_sub(bp, vecs_sb[:, bcol:bcol + 1], m2)
                return a, bp

            # ---- BN1 (combine + prescale W1) ----
            nc.vector.bn_aggr(mv1[:], sb1[:])
            a1, b1p = bn_round(mv1, G1, "cc1", 0, 1, 0)
            nc.gpsimd.tensor_scalar_mul(fW1s_sb[:], fW1f_sb[:], a1)
            nc.vector.tensor_copy(b1pb[:], b1p)
            pfb = ps.tile([128, 4], f32, tag="po", bufs=1, name="pfb")
            for qf in range(4):
                nc.tensor.matmul(pfb[:, qf:qf + 1],
                                 lhsT=fW1b_sb[:, 128 * qf:128 * (qf + 1)],
                                 rhs=b1pb[:], start=True, stop=True)
            nc.vector.tensor_add(biasq[:], pfb[:], vecs_sb[:, 5:9])
            for c in range(4):
                nc.gpsimd.tensor_scalar(
                    h1nT[:, 512 * c:512 * (c + 1)],
                    h1T[:, 512 * c:512 * (c + 1)],
                    a1, b1p, op0=Alu.mult, op1=Alu.add)

            # ---- FFN (on unnormalized h1 with prescaled weights) ----
            h1r = h1T

            def ffn1(c):
                for half in range(2):
                    pf = ps.tile([128, 1024], f32, tag="sc", bufs=2,
                                 name=f"pf{c}{half}")
                    for j in range(2):
                        qf = 2 * half + j
                        nc.tensor.matmul(
                            pf[:, 512 * j:512 * (j + 1)],
                            lhsT=fW1s_sb[:, 128 * qf:128 * (qf + 1)],
                            rhs=h1r[:, 512 * c:512 * (c + 1)],
                            start=True, stop=True)
                    for j in range(2):
                        qf = 2 * half + j
                        src = pf[:, 512 * j:512 * (j + 1)]
                        dst = h2T[qf][:, 512 * c:512 * (c + 1)]
                        if qf % 2 == 0:
                            nc.scalar.activation(dst, src, Act.Relu,
                                                 bias=biasq[:, qf:qf + 1])
                        else:
                            nc.vector.tensor_scalar(
                                dst, src, biasq[:, qf:qf + 1], 0.0,
                                op0=Alu.add, op1=Alu.max)

            def ffn2(c):
                p2 = ps.tile([128, 512], f32, tag="av", bufs=1, name=f"p2{c}")
                for qf in range(4):
                    nc.tensor.matmul(p2[:],
                                     lhsT=fW2q_sb[:, 128 * qf:128 * (qf + 1)],
                                     rhs=h2T[qf][:, 512 * c:512 * (c + 1)],
                                     start=(qf == 0), stop=(qf == 3))
                nc.vector.tensor_add(yT[:, 512 * c:512 * (c + 1)], p2[:],
                                     h1nT[:, 512 * c:512 * (c + 1)])
                nc.vector.bn_stats(sb2[:, 6 * c:6 * (c + 1)],
                                   yT[:, 512 * c:512 * (c + 1)])

            ffn1(0)
            ffn1(1)
            ffn2(0)
            ffn1(2)
            ffn2(1)
            ffn1(3)
            ffn2(2)
            ffn2(3)

            # ---- BN2 + output ----
            nc.vector.bn_aggr(mv2[:], sb2[:])
            a2, b2p = bn_round(mv2, G2, "cc2", 2, 3, 12)
            for c in range(4):
                nc.vector.tensor_scalar(
                    sqo[:, 512 * c:512 * (c + 1)],
                    yT[:, 512 * c:512 * (c + 1)],
                    a2, b2p, op0=Alu.mult, op1=Alu.add)
                nc.sync.dma_start(d_yT[:, 512 * c:512 * (c + 1)],
                                  sqo[:, 512 * c:512 * (c + 1)])

    nc.compile()
    return nc


def _host_prep(inputs):
    f = np.float32
    Wq, Wk, Wv, Wo = (np.asarray(inputs[k], f) for k in ("Wq", "Wk", "Wv", "Wo"))
    WqQ = np.zeros((E, 256), f)
    WkQ = np.zeros((E, 256), f)
    for g in range(2):
        for hh in range(4):
            h = 4 * g + hh
            WqQ[:, 128 * g + 32 * hh:128 * g + 32 * hh + 16] = Wq[h]
            WkQ[:, 128 * g + 32 * hh:128 * g + 32 * hh + 16] = Wk[h]
    WvA = np.ascontiguousarray(np.transpose(Wv, (1, 0, 2)).reshape(E, H * KD))
    WoA = np.ascontiguousarray(Wo.reshape(H * KD, E))
    We1a = np.zeros((3, E), f)
    We1a[:2] = np.asarray(inputs["We1"], f)
    We1a[2] = np.asarray(inputs["be1"], f)
    fW1 = np.asarray(inputs["ffW1"], f)
    fW2q = np.ascontiguousarray(
        np.asarray(inputs["ffW2"], f).reshape(4, 128, E).transpose(1, 0, 2))
    vecs = np.zeros((128, 12), f)
    vecs[:, 0] = inputs["bn1_w"]
    vecs[:, 1] = inputs["bn1_b"]
    vecs[:, 2] = inputs["bn2_w"]
    vecs[:, 3] = inputs["bn2_b"]
    vecs[:, 4] = EPS
    vecs[:, 5:9] = np.asarray(inputs["ffb1"], f).reshape(4, 128).T
    import ml_dtypes
    bf = ml_dtypes.bfloat16
    return {
        "We1a": We1a,
        "WqQ": WqQ,
        "WkQ": WkQ,
        "WvA": WvA.astype(bf),
        "WoA": WoA.astype(bf),
        "fW1f": fW1,
        "fW1b": fW1.astype(bf),
        "fW2q": np.ascontiguousarray(fW2q.reshape(128, 512)).astype(bf),
        "ident": np.eye(128, dtype=f).astype(bf),
        "vecs": vecs,
    }


def _get_runner():
    """Build the sharded jitted executable once and cache it."""
    if "runner" in _CACHE:
        return _CACHE["runner"]
    import jax
    import concourse.mybir as mybir
    from jax.sharding import Mesh, PartitionSpec
    from jax.experimental.shard_map import shard_map
    from concourse.bass2jax import (_bass_exec_p, install_neuronx_cc_hook,
                                    partition_id_tensor)

    if "nc" not in _CACHE:
        _CACHE["nc"] = _build_nc()
    nc = _CACHE["nc"]
    install_neuronx_cc_hook()
    assert nc.dbg_addr is None

    partition_name = (nc.partition_id_tensor.name
                      if nc.partition_id_tensor else None)
    in_names, out_names, out_avals, zero_outs = [], [], [], []
    for alloc in nc.m.functions[0].allocations:
        if not isinstance(alloc, mybir.MemoryLocationSet):
            continue
        name = alloc.memorylocations[0].name
        if alloc.kind == "ExternalInput":
            if name != partition_name:
                in_names.append(name)
        elif alloc.kind == "ExternalOutput":
            shape = tuple(alloc.tensor_shape)
            dtype = mybir.dt.np(alloc.dtype)
            out_names.append(name)
            out_avals.append(jax.core.ShapedArray(shape, dtype))
            zero_outs.append(np.zeros(shape, dtype))
    n_params = len(in_names)
    n_outs = len(out_avals)
    all_in_names = list(in_names) + list(out_names)
    if partition_name is not None:
        all_in_names.append(partition_name)
    donate = tuple(range(n_params, n_params + n_outs))

    def _body(*args):
        operands = list(args)
        if partition_name is not None:
            operands.append(partition_id_tensor())
        outs = _bass_exec_p.bind(
            *operands,
            out_avals=tuple(out_avals),
            in_names=tuple(all_in_names),
            out_names=tuple(out_names),
            lowering_input_output_aliases=(),
            sim_require_finite=True,
            sim_require_nnan=True,
            nc=nc,
        )
        return tuple(outs)

    devices = jax.devices()[:NCORES]
    mesh = Mesh(np.asarray(devices), ("core",))
    in_specs = (PartitionSpec("core"),) * (n_params + n_outs)
    out_specs = (PartitionSpec("core"),) * len(out_names)
    sharded = jax.jit(
        shard_map(_body, mesh=mesh, in_specs=in_specs, out_specs=out_specs,
                  check_rep=False),
        donate_argnums=donate, keep_unused=True)

    def run(in_maps):
        per_core = [[np.asarray(m[name]) for name in in_names]
                    for m in in_maps]
        concat_in = [np.concatenate([per_core[c][i] for c in range(NCORES)],
                                    axis=0) for i in range(n_params)]
        concat_zeros = [np.zeros((NCORES * z.shape[0], *z.shape[1:]), z.dtype)
                        for z in zero_outs]
        out_arrs = sharded(*concat_in, *concat_zeros)
        out_arrs = [np.asarray(a) for a in out_arrs]
        return [{name: out_arrs[i].reshape(NCORES, *out_avals[i].shape)[c]
                 for i, name in enumerate(out_names)}
                for c in range(NCORES)]

    _CACHE["runner"] = run
    return run


def _make_in_maps(inputs):
    shared = _host_prep(inputs)
    x1 = np.asarray(inputs["x1"], np.float32)
    in_maps = []
    for cidx in range(NCORES):
        m = dict(shared)
        xl = x1[BPC * cidx:BPC * (cidx + 1)].reshape(T, D_IN)
        xa = np.ones((3, T), np.float32)
        xa[:2] = xl.T
        m["xTa"] = xa
        in_maps.append(m)
    return in_maps


def kernel(**inputs):
    run = _get_runner()
    results = run(_make_in_maps(inputs))
    outs = []
    for cidx in range(NCORES):
        yTo = results[cidx]["yT"]          # [E, T]
        outs.append(np.ascontiguousarray(yTo.T).reshape(BPC, N, E))
    return np.concatenate(outs, 0).astype(np.float32)


# revision 28
# speedup vs baseline: 1.6841x; 1.0795x over previous
"""Trainium2 Bass kernel for nn_MHABlock (dense transformer block).

Sharding: data-parallel over batch - 8 cores x 4 batches (2048 tokens/core).
BatchNorm stats are exact via two AllGather collectives of per-core
(mean, var) pairs ([128,2] each) combined locally on every core.

Layout: E-major ("T" = [E(128 partitions), tokens(free)]).  Attention uses
the scoresT formulation (scores [k_tok, q_tok]) for exp, but attnV is
*flipped*: out[q_tok, v] with N=17 per matmul (16 v dims + a ones column of
V giving the softmax denominator per q partition).  Normalization is then a
per-partition reciprocal + one broadcast multiply on DVE; heads are
transposed back to (h,v)-major with PE transpose-mode and fed to the output
projection.  Softmax exp is split between the ACT engine (table exp) and the
DVE (Schraudolph bf16-bit exp via f32->int16 convert).  PSUM->SBUF moves
that need no dtype change (H0, q/k in f32->f32r, transposed heads) go over
DMA engines instead of DVE/ACT.  FFN consumes unnormalized h1 with
BN1-scaled weights (a*W1) so the BN1 apply is off the critical path.
"""

import numpy as np

B, N, D_IN, E, H, KD, FF = 32, 512, 2, 128, 8, 16, 512
NCORES = 8
BPC = B // NCORES          # batches per core
T = BPC * N                # 2048 local tokens
NTOK = B * N               # global token count for BN
NORM = 1.0 / np.sqrt(16.0)
EPS = 1e-5

# Schraudolph bf16-bit exp:  i16 = trunc(scores * SCH_A + SCH_B),
# bits reinterpreted as bf16 ~= exp(NORM * scores).  Calibrated for the
# truncating f32->int16 convert (C=6.9).
SCH_A = float(0.25 * 128 * np.log2(np.e))
SCH_B = 16256.0 - 6.9
# per-batch exp tile indices (of 16) computed on DVE instead of ACT
DVE_EXP = frozenset({2, 6, 10, 14})

_CACHE = {}
LAST_RESULT = None


def _build_nc():
    import concourse.bass as bass  # noqa: F401
    import concourse.mybir as mybir
    import concourse.tile as tile
    from concourse import bacc

    f32 = mybir.dt.float32
    f32r = mybir.dt.float32r
    bf16 = mybir.dt.bfloat16
    i16 = mybir.dt.int16
    Act = mybir.ActivationFunctionType
    Alu = mybir.AluOpType
    AX = mybir.AxisListType

    nc = bacc.Bacc("TRN2", target_bir_lowering=False, debug=False,
                   enable_asserts=False, num_devices=NCORES)

    # ---- DRAM I/O ----
    d_xW = nc.dram_tensor("xW", [3, T + E], f32r, kind="ExternalInput").ap()
    d_WqQ = nc.dram_tensor("WqQ", [E, 256], f32r, kind="ExternalInput").ap()
    d_WkQ = nc.dram_tensor("WkQ", [E, 256], f32r, kind="ExternalInput").ap()
    d_WvA = nc.dram_tensor("WvA", [E, 128], bf16, kind="ExternalInput").ap()
    d_WoA = nc.dram_tensor("WoA", [128, E], bf16, kind="ExternalInput").ap()
    d_fW1f = nc.dram_tensor("fW1f", [E, FF], f32, kind="ExternalInput").ap()
    d_fW1b = nc.dram_tensor("fW1b", [E, FF], bf16, kind="ExternalInput").ap()
    d_fW2q = nc.dram_tensor("fW2q", [128, 512], bf16, kind="ExternalInput").ap()
    d_ident = nc.dram_tensor("ident", [128, 128], bf16, kind="ExternalInput").ap()
    d_vecs = nc.dram_tensor("vecs", [128, 12], f32, kind="ExternalInput").ap()
    d_yT = nc.dram_tensor("yT", [E, T], f32, kind="ExternalOutput").ap()

    RG = [list(range(NCORES))]

    with tile.TileContext(nc) as tc:
        with tc.sbuf_pool(name="sb", bufs=1) as sb, \
             tc.psum_pool(name="ps", bufs=1) as ps, \
             tc.tile_pool(name="dr", bufs=1, space="DRAM") as dr:

            def P(shape, dt, name):  # persistent tile
                return sb.tile(shape, dt, name=name, tag=name, bufs=1)

            xW = P([3, T + E], f32r, "xW_sb")
            xTa = xW[:, :T]
            We1a_sb = xW[:, T:]
            WqQ_sb = P([128, 256], f32r, "WqQ_sb")
            WkQ_sb = P([128, 256], f32r, "WkQ_sb")
            WvA_sb = P([128, 128], bf16, "WvA_sb")
            WoA_sb = P([128, 128], bf16, "WoA_sb")
            fW1f_sb = P([128, FF], f32, "fW1f_sb")
            fW1b_sb = P([128, FF], bf16, "fW1b_sb")
            fW1s_sb = P([128, FF], f32r, "fW1s_sb")
            fW2q_sb = P([128, 512], bf16, "fW2q_sb")
            ident_sb = P([128, 128], bf16, "ident_sb")
            vecs_sb = P([128, 12], f32, "vecs_sb")

            H0T = P([128, T], f32r, "H0T")
            H0b = P([128, T], bf16, "H0b")
            qT = [P([128, T], f32r, f"qT{g}") for g in range(2)]
            kT = [P([128, T], f32r, f"kT{g}") for g in range(2)]
            V17 = P([128, 16 * 136], bf16, "V17")
            h1T = P([128, T], f32r, "h1T")
            h1nT = P([128, T], f32, "h1nT")
            yT = P([128, T], f32, "yT_sb")
            sqo = P([128, T], f32, "sqo")
            h2T = [P([128, T], bf16, f"h2T{qf}") for qf in range(4)]
            hTs = [P([128, 512], bf16, f"hTs{b}") for b in range(4)]
            sb1 = P([128, 24], f32, "sb1")
            sb2 = P([128, 24], f32, "sb2")
            mv1 = P([128, 2], f32, "mv1")
            mv2 = P([128, 2], f32, "mv2")
            G1 = P([128, 16], f32, "G1")
            G2 = P([128, 16], f32, "G2")
            cmb = P([128, 24], f32, "cmb")
            biasq = P([128, 4], f32, "biasq")
            b1pb = P([128, 1], bf16, "b1pb")

            # ---- load inputs (xTa/We1a first: they gate the embedding) ----
            nc.sync.dma_start(xW[:], d_xW)
            nc.sync.dma_start(WqQ_sb[:], d_WqQ)
            nc.sync.dma_start(WkQ_sb[:], d_WkQ)
            nc.sync.dma_start(WvA_sb[:], d_WvA)
            nc.sync.dma_start(WoA_sb[:], d_WoA)
            nc.sync.dma_start(ident_sb[:], d_ident)
            nc.sync.dma_start(vecs_sb[:], d_vecs)
            nc.sync.dma_start(fW2q_sb[:], d_fW2q)
            nc.sync.dma_start(fW1f_sb[:], d_fW1f)
            nc.sync.dma_start(fW1b_sb[:], d_fW1b)

            # ones columns of V17 (col 16 of every 17-block)
            v17v = V17.rearrange("p (t h s) -> p t h s", t=16, h=8)
            nc.gpsimd.memset(v17v[:, :, :, 16:17], 1.0)

            # pin the first ACT table load to the exp set
            nc.scalar.activation(cmb[:, 22:23], vecs_sb[:, 4:5], Act.Exp)

            # ---- Phase A: embedding  h0 = [x|1] @ [We1;be1]  (E-major) ----
            for cp in range(2):
                pm = ps.tile([128, 1024], f32, tag="sc", bufs=2, name=f"pm{cp}")
                for j in range(2):
                    c = 2 * cp + j
                    nc.tensor.matmul(pm[:, 512 * j:512 * (j + 1)],
                                     lhsT=We1a_sb,
                                     rhs=xTa[:, 512 * c:512 * (c + 1)],
                                     start=True, stop=True)
                nc.scalar.copy(H0T[:, 1024 * cp:1024 * (cp + 1)], pm[:])
                nc.gpsimd.tensor_copy(H0b[:, 1024 * cp:1024 * (cp + 1)],
                                      H0T[:, 1024 * cp:1024 * (cp + 1)])

            H0r = H0T

            # ---- Phase B: q/k projections (quad-padded, f32 via DMA) ----
            for g in range(2):
                for cp in range(2):
                    pq = ps.tile([128, 1024], f32, tag="sc", bufs=2,
                                 name=f"pq{g}{cp}")
                    pk = ps.tile([128, 1024], f32, tag="sc", bufs=2,
                                 name=f"pk{g}{cp}")
                    for j in range(2):
                        c = 2 * cp + j
                        nc.tensor.matmul(pq[:, 512 * j:512 * (j + 1)],
                                         lhsT=WqQ_sb[:, 128 * g:128 * (g + 1)],
                                         rhs=H0r[:, 512 * c:512 * (c + 1)],
                                         start=True, stop=True)
                        nc.tensor.matmul(pk[:, 512 * j:512 * (j + 1)],
                                         lhsT=WkQ_sb[:, 128 * g:128 * (g + 1)],
                                         rhs=H0r[:, 512 * c:512 * (c + 1)],
                                         start=True, stop=True)
                    nc.vector.tensor_copy(qT[g][:, 1024 * cp:1024 * (cp + 1)],
                                          pq[:])
                    nc.scalar.copy(kT[g][:, 1024 * cp:1024 * (cp + 1)], pk[:])

            # ---- Phase C: v projection (emitted per-batch inside Phase D) --
            def emit_vproj(t):
                pv = ps.tile([128, 128], f32, tag="tp", bufs=1, name=f"pv{t}")
                nc.tensor.matmul(pv[:], lhsT=H0b[:, 128 * t:128 * (t + 1)],
                                 rhs=WvA_sb[:], start=True, stop=True)
                dst = v17v[:, t, :, 0:16]
                srcv = pv.rearrange("p (h s) -> p h s", h=8)
                if t % 2 == 0:
                    nc.vector.tensor_copy(dst, srcv)
                else:
                    nc.scalar.copy(dst, srcv)

            # ---- Phase D: attention, software-pipelined over batches ----
            ex_tiles = {}
            av_tiles = {}

            def score_mm(dst, b, h, c):
                g, hh = h // 4, h % 4
                nc.tensor.matmul(
                    dst,
                    lhsT=kT[g][32 * hh:32 * (hh + 1),
                               512 * b + 128 * c:512 * b + 128 * (c + 1)],
                    rhs=qT[g][32 * hh:32 * (hh + 1),
                              512 * b:512 * (b + 1)],
                    start=True, stop=True,
                    tile_position=(32 * hh, 0))

            def emit_scores_exp(b, h):
                for cp in range(2):
                    ex = sb.tile([128, 1024], bf16, tag="ex", bufs=16,
                                 name=f"ex{b}{h}{cp}")
                    if 2 * h + cp in DVE_EXP:
                        # DVE-exp tiles run on their own 1-bank rotation so
                        # they never bubble the ACT pipeline
                        for j in range(2):
                            scd = ps.tile([128, 512], f32, tag="scd", bufs=1,
                                          name=f"scd{b}{h}{cp}{j}")
                            score_mm(scd[:], b, h, 2 * cp + j)
                            nc.vector.tensor_scalar(
                                ex.bitcast(i16)[:, 512 * j:512 * (j + 1)],
                                scd[:], SCH_A, SCH_B,
                                op0=Alu.mult, op1=Alu.add)
                    else:
                        scp = ps.tile([128, 1024], f32, tag="sc", bufs=2,
                                      name=f"scp{b}{h}{cp}")
                        for j in range(2):
                            score_mm(scp[:, 512 * j:512 * (j + 1)], b, h,
                                     2 * cp + j)
                        nc.scalar.activation(ex[:], scp[:], Act.Exp,
                                             scale=float(NORM))
                    ex_tiles[(b, h, cp)] = ex

            def emit_attnv(b, h):
                # attnV: out [q, 17] per (h, qc), accumulated over 4 k-chunks
                if b not in av_tiles:
                    av_tiles[b] = ps.tile([128, 1024], f32, tag="av", bufs=1,
                                          name=f"av{b}")
                av = av_tiles[b]
                for qc in range(4):
                    for kc in range(4):
                        cp, j = kc // 2, kc % 2
                        ex = ex_tiles[(b, h, cp)]
                        nc.tensor.matmul(
                            av[:, 256 * qc + 17 * h:256 * qc + 17 * h + 17],
                            lhsT=ex[:, 512 * j + 128 * qc:
                                    512 * j + 128 * (qc + 1)],
                            rhs=V17[:, 136 * (4 * b + kc) + 17 * h:
                                    136 * (4 * b + kc) + 17 * (h + 1)],
                            start=(kc == 0), stop=(kc == 3))

            def emit_attn_tail(b):
                av = av_tiles[b]
                # normalize by the ones-column sums (per q partition)
                av4 = av.rearrange("p (q x) -> p q x", q=4)[:, :, :136]
                av5 = av4.rearrange("p q (h s) -> p q h s", h=8)
                rd = sb.tile([128, 32], f32, tag="rd", bufs=2, name=f"rd{b}")
                rd4 = rd.rearrange("p (q h s) -> p q h s", q=4, h=8)
                nc.vector.reciprocal(rd4, av5[:, :, :, 16:17])
                hn = sb.tile([128, 512], bf16, tag="hn", bufs=2, name=f"hn{b}")
                hn4 = hn.rearrange("p (q h s) -> p q h s", q=4, h=8)
                vals = av5[:, :, :, 0:16]
                rdb = bass.broadcast_tensor_aps(vals, rd4)[1]
                nc.vector.tensor_mul(hn4, vals, rdb)
                # transpose back to (h,v)-major and project
                tp = ps.tile([128, 512], bf16, tag="tp", bufs=1, name=f"tp{b}")
                for qc in range(4):
                    nc.tensor.transpose(tp[:, 128 * qc:128 * (qc + 1)],
                                        hn[:, 128 * qc:128 * (qc + 1)],
                                        ident_sb[:])
                nc.vector.tensor_copy(hTs[b][:], tp[:])
                po = ps.tile([128, 512], f32, tag="tp", bufs=1, name=f"po{b}")
                nc.tensor.matmul(po[:], lhsT=WoA_sb[:], rhs=hTs[b][:],
                                 start=True, stop=True)
                nc.vector.tensor_add(h1T[:, 512 * b:512 * (b + 1)], po[:],
                                     H0T[:, 512 * b:512 * (b + 1)])
                nc.vector.bn_stats(sb1[:, 6 * b:6 * (b + 1)],
                                   h1T[:, 512 * b:512 * (b + 1)])

            for b in range(5):
                for h in range(8):
                    if b < 4 and h < 4:
                        emit_vproj(4 * b + h)
                    if b > 0:
                        emit_attnv(b - 1, h)
                    if b < 4:
                        emit_scores_exp(b, h)
                if b > 0:
                    emit_attn_tail(b - 1)

            # ---- cross-core BN stats helper ----
            def bn_round(mv, Gt, ccname, wcol, bcol, col0):
                """AllGather per-core (mean,var); combine; produce
                a = w*rstd (cmb col0), bp = b - mean*a (col0+1)."""
                cc_in = dr.tile([128, 2], f32, name=f"{ccname}_in",
                                tag=f"{ccname}_in")
                cc_out = dr.tile([NCORES, 128, 2], f32, addr_space="Shared",
                                 name=f"{ccname}_out", tag=f"{ccname}_out")
                nc.sync.dma_start(cc_in[:], mv[:])
                nc.gpsimd.collective_compute(
                    "AllGather", Alu.bypass, replica_groups=RG,
                    ins=[cc_in[:]], outs=[cc_out[:]])
                nc.sync.dma_start(
                    Gt.rearrange("p (r s) -> p r s", r=8),
                    cc_out.rearrange("r p s -> p r s"))
                Gr = Gt.rearrange("p (r s) -> p r s", r=8)   # [128, 8, 2]
                G3 = Gr.rearrange("p r s -> p s r")          # [128, 2, 8] view
                c = cmb
                msq = c[:, col0 + 2:col0 + 10].rearrange(
                    "p (a r) -> p a r", a=1)
                nc.vector.tensor_mul(msq, G3[:, 0:1, :], G3[:, 0:1, :])
                nc.vector.tensor_add(G3[:, 1:2, :], G3[:, 1:2, :], msq)
                me2 = c[:, col0 + 4:col0 + 6]                # [mean, E2]*8
                nc.vector.reduce_sum(
                    out=me2.rearrange("p (a s) -> p a s", a=2),
                    in_=G3, axis=AX.X)
                nc.vector.tensor_scalar_mul(me2, me2, 0.125)
                mean = c[:, col0 + 4:col0 + 5]
                e2 = c[:, col0 + 5:col0 + 6]
                m2 = c[:, col0 + 6:col0 + 7]
                nc.vector.tensor_mul(m2, mean, mean)
                var = c[:, col0 + 7:col0 + 8]
                nc.vector.tensor_sub(var, e2, m2)
                sd = c[:, col0 + 8:col0 + 9]
                nc.scalar.activation(sd, var, Act.Sqrt, bias=vecs_sb[:, 4:5])
                rstd = c[:, col0 + 9:col0 + 10]
                nc.vector.reciprocal(rstd, sd)
                a = c[:, col0:col0 + 1]
                nc.vector.tensor_mul(a, rstd, vecs_sb[:, wcol:wcol + 1])
                nc.vector.tensor_mul(m2, mean, a)
                bp = c[:, col0 + 1:col0 + 2]
                nc.vector.tensor_sub(bp, vecs_sb[:, bcol:bcol + 1], m2)
                return a, bp

            # warm the sqrt table set while the collective runs (input is
            # batch-3's bn_stats count column, so this can't be hoisted into
            # the exp stream by the scheduler)
            nc.scalar.activation(cmb[:, 23:24], sb1[:, 18:19], Act.Sqrt)

            # ---- BN1 (combine + prescale W1) ----
            nc.vector.bn_aggr(mv1[:], sb1[:])
            a1, b1p = bn_round(mv1, G1, "cc1", 0, 1, 0)
            nc.vector.tensor_scalar_mul(fW1s_sb[:], fW1f_sb[:], a1)
            nc.vector.tensor_copy(b1pb[:], b1p)
            pfb = ps.tile([128, 4], f32, tag="tp", bufs=1, name="pfb")
            for qf in range(4):
                nc.tensor.matmul(pfb[:, qf:qf + 1],
                                 lhsT=fW1b_sb[:, 128 * qf:128 * (qf + 1)],
                                 rhs=b1pb[:], start=True, stop=True)
            nc.vector.tensor_add(biasq[:], pfb[:], vecs_sb[:, 5:9])
            for c in range(4):
                nc.gpsimd.tensor_scalar(
                    h1nT[:, 512 * c:512 * (c + 1)],
                    h1T[:, 512 * c:512 * (c + 1)],
                    a1, b1p, op0=Alu.mult, op1=Alu.add)

            # ---- FFN (on unnormalized h1 with prescaled weights) ----
            h1r = h1T

            def ffn1(c):
                for half in range(2):
                    pf = ps.tile([128, 1024], f32, tag="sc", bufs=2,
                                 name=f"pf{c}{half}")
                    for j in range(2):
                        qf = 2 * half + j
                        nc.tensor.matmul(
                            pf[:, 512 * j:512 * (j + 1)],
                            lhsT=fW1s_sb[:, 128 * qf:128 * (qf + 1)],
                            rhs=h1r[:, 512 * c:512 * (c + 1)],
                            start=True, stop=True)
                    for j in range(2):
                        qf = 2 * half + j
                        src = pf[:, 512 * j:512 * (j + 1)]
                        dst = h2T[qf][:, 512 * c:512 * (c + 1)]
                        if qf % 2 == 0:
                            nc.scalar.activation(dst, src, Act.Relu,
                                                 bias=biasq[:, qf:qf + 1])
                        else:
                            nc.vector.tensor_scalar(
                                dst, src, biasq[:, qf:qf + 1], 0.0,
                                op0=Alu.add, op1=Alu.max)

            def ffn2(c):
                p2 = ps.tile([128, 512], f32, tag="av", bufs=1, name=f"p2{c}")
                for qf in range(4):
                    nc.tensor.matmul(p2[:],
                                     lhsT=fW2q_sb[:, 128 * qf:128 * (qf + 1)],
                                     rhs=h2T[qf][:, 512 * c:512 * (c + 1)],
                                     start=(qf == 0), stop=(qf == 3))
                nc.vector.tensor_add(yT[:, 512 * c:512 * (c + 1)], p2[:],
                                     h1nT[:, 512 * c:512 * (c + 1)])
                nc.vector.bn_stats(sb2[:, 6 * c:6 * (c + 1)],
                                   yT[:, 512 * c:512 * (c + 1)])

            ffn1(0)
            ffn1(1)
            ffn2(0)
            ffn1(2)
            ffn2(1)
            ffn1(3)
            ffn2(2)
            ffn2(3)

            # ---- BN2 + output ----
            nc.vector.bn_aggr(mv2[:], sb2[:])
            a2, b2p = bn_round(mv2, G2, "cc2", 2, 3, 12)
            for c in range(4):
                veng = nc.vector if c % 2 == 0 else nc.gpsimd
                veng.tensor_scalar(
                    sqo[:, 512 * c:512 * (c + 1)],
                    yT[:, 512 * c:512 * (c + 1)],
                    a2, b2p, op0=Alu.mult, op1=Alu.add)
                nc.sync.dma_start(d_yT[:, 512 * c:512 * (c + 1)],
                                  sqo[:, 512 * c:512 * (c + 1)])

    nc.compile()
    return nc


def _host_prep(inputs):
    f = np.float32
    Wq, Wk, Wv, Wo = (np.asarray(inputs[k], f) for k in ("Wq", "Wk", "Wv", "Wo"))
    WqQ = np.zeros((E, 256), f)
    WkQ = np.zeros((E, 256), f)
    for g in range(2):
        for hh in range(4):
            h = 4 * g + hh
            WqQ[:, 128 * g + 32 * hh:128 * g + 32 * hh + 16] = Wq[h]
            WkQ[:, 128 * g + 32 * hh:128 * g + 32 * hh + 16] = Wk[h]
    WvA = np.ascontiguousarray(np.transpose(Wv, (1, 0, 2)).reshape(E, H * KD))
    WoA = np.ascontiguousarray(Wo.reshape(H * KD, E))
    We1a = np.zeros((3, E), f)
    We1a[:2] = np.asarray(inputs["We1"], f)
    We1a[2] = np.asarray(inputs["be1"], f)
    # We1a is appended to each core's x block as cols [T:T+E] of xW
    fW1 = np.asarray(inputs["ffW1"], f)
    fW2q = np.ascontiguousarray(
        np.asarray(inputs["ffW2"], f).reshape(4, 128, E).transpose(1, 0, 2))
    vecs = np.zeros((128, 12), f)
    vecs[:, 0] = inputs["bn1_w"]
    vecs[:, 1] = inputs["bn1_b"]
    vecs[:, 2] = inputs["bn2_w"]
    vecs[:, 3] = inputs["bn2_b"]
    vecs[:, 4] = EPS
    vecs[:, 5:9] = np.asarray(inputs["ffb1"], f).reshape(4, 128).T
    import ml_dtypes
    bf = ml_dtypes.bfloat16
    return {
        "We1a_block": We1a,
        "WqQ": WqQ,
        "WkQ": WkQ,
        "WvA": WvA.astype(bf),
        "WoA": WoA.astype(bf),
        "fW1f": fW1,
        "fW1b": fW1.astype(bf),
        "fW2q": np.ascontiguousarray(fW2q.reshape(128, 512)).astype(bf),
        "ident": np.eye(128, dtype=f).astype(bf),
        "vecs": vecs,
    }


def _get_runner():
    """Build the sharded jitted executable once and cache it."""
    if "runner" in _CACHE:
        return _CACHE["runner"]
    import jax
    import concourse.mybir as mybir
    from jax.sharding import Mesh, PartitionSpec
    from jax.experimental.shard_map import shard_map
    from concourse.bass2jax import (_bass_exec_p, install_neuronx_cc_hook,
                                    partition_id_tensor)

    if "nc" not in _CACHE:
        _CACHE["nc"] = _build_nc()
    nc = _CACHE["nc"]
    install_neuronx_cc_hook()
    assert nc.dbg_addr is None

    partition_name = (nc.partition_id_tensor.name
                      if nc.partition_id_tensor else None)
    in_names, out_names, out_avals, zero_outs = [], [], [], []
    for alloc in nc.m.functions[0].allocations:
        if not isinstance(alloc, mybir.MemoryLocationSet):
            continue
        name = alloc.memorylocations[0].name
        if alloc.kind == "ExternalInput":
            if name != partition_name:
                in_names.append(name)
        elif alloc.kind == "ExternalOutput":
            shape = tuple(alloc.tensor_shape)
            dtype = mybir.dt.np(alloc.dtype)
            out_names.append(name)
            out_avals.append(jax.core.ShapedArray(shape, dtype))
            zero_outs.append(np.zeros(shape, dtype))
    n_params = len(in_names)
    n_outs = len(out_avals)
    all_in_names = list(in_names) + list(out_names)
    if partition_name is not None:
        all_in_names.append(partition_name)
    donate = tuple(range(n_params, n_params + n_outs))

    def _body(*args):
        operands = list(args)
        if partition_name is not None:
            operands.append(partition_id_tensor())
        outs = _bass_exec_p.bind(
            *operands,
            out_avals=tuple(out_avals),
            in_names=tuple(all_in_names),
            out_names=tuple(out_names),
            lowering_input_output_aliases=(),
            sim_require_finite=True,
            sim_require_nnan=True,
            nc=nc,
        )
        return tuple(outs)

    devices = jax.devices()[:NCORES]
    mesh = Mesh(np.asarray(devices), ("core",))
    in_specs = (PartitionSpec("core"),) * (n_params + n_outs)
    out_specs = (PartitionSpec("core"),) * len(out_names)
    sharded = jax.jit(
        shard_map(_body, mesh=mesh, in_specs=in_specs, out_specs=out_specs,
                  check_rep=False),
        donate_argnums=donate, keep_unused=True)

    def run(in_maps):
        per_core = [[np.asarray(m[name]) for name in in_names]
                    for m in in_maps]
        concat_in = [np.concatenate([per_core[c][i] for c in range(NCORES)],
                                    axis=0) for i in range(n_params)]
        concat_zeros = [np.zeros((NCORES * z.shape[0], *z.shape[1:]), z.dtype)
                        for z in zero_outs]
        out_arrs = sharded(*concat_in, *concat_zeros)
        out_arrs = [np.asarray(a) for a in out_arrs]
        return [{name: out_arrs[i].reshape(NCORES, *out_avals[i].shape)[c]
                 for i, name in enumerate(out_names)}
                for c in range(NCORES)]

    _CACHE["runner"] = run
    return run


def _make_in_maps(inputs):
    shared = _host_prep(inputs)
    x1 = np.asarray(inputs["x1"], np.float32)
    in_maps = []
    for cidx in range(NCORES):
        m = dict(shared)
        We1a = m.pop("We1a_block")
        xl = x1[BPC * cidx:BPC * (cidx + 1)].reshape(T, D_IN)
        xa = np.ones((3, T + E), np.float32)
        xa[:2, :T] = xl.T
        xa[:, T:] = We1a
        m["xW"] = xa
        in_maps.append(m)
    return in_maps


def kernel(**inputs):
    run = _get_runner()
    results = run(_make_in_maps(inputs))
    outs = []
    for cidx in range(NCORES):
        yTo = results[cidx]["yT"]          # [E, T]
        outs.append(np.ascontiguousarray(yTo.T).reshape(BPC, N, E))
    return np.concatenate(outs, 0).astype(np.float32)


# revision 50
# speedup vs baseline: 1.7576x; 1.0437x over previous
"""Trainium2 Bass kernel for nn_MHABlock (dense transformer block).

Sharding: data-parallel over batch - 8 cores x 4 batches (2048 tokens/core).
BatchNorm stats are exact via two AllGather collectives of per-core
(mean, var) pairs ([128,2] each) combined locally on every core.

Layout: E-major ("T" = [E(128 partitions), tokens(free)]).  Attention uses
the scoresT formulation (scores [k_tok, q_tok]) for exp, but attnV is
*flipped*: out[q_tok, 17] per (head, q-chunk) matmul (16 v dims + a ones
column of V giving the softmax denominator per q partition, so PE pays N=17
instead of N=512 per accumulation step).  Normalization is a per-partition
reciprocal + one stride-0-broadcast multiply on DVE; heads are transposed
back to (h,v)-major with PE transpose-mode and fed to the output projection.
Softmax exp is split ~11:5 per batch between the ACT engine (table exp,
[128,1024] tiles double-buffered in PSUM) and the DVE (Schraudolph bf16-bit
exp: i16 = trunc(A*s + B) bit-cast to bf16, on a decoupled 1-bank PSUM
rotation so DVE tiles never bubble the ACT pipeline).  Embedding and q/k
projections run in float32r (full-rate fp32 matmul for N>=512; exact f32
scores).  FFN consumes unnormalized h1 (f32r) with BN1-prescaled weights
(a1*W1) so the BN1 apply stays off the critical path; ffb2 cancels in BN2.
Cross-core BN stats ride two AllGathers of per-core (mean, var) combined
locally; the sqrt table-set load is prefetched under the first collective.
"""

import numpy as np

B, N, D_IN, E, H, KD, FF = 32, 512, 2, 128, 8, 16, 512
NCORES = 8
BPC = B // NCORES          # batches per core
T = BPC * N                # 2048 local tokens
NTOK = B * N               # global token count for BN
NORM = 1.0 / np.sqrt(16.0)
EPS = 1e-5

# Schraudolph bf16-bit exp:  i16 = trunc(scores * SCH_A + SCH_B),
# bits reinterpreted as bf16 ~= exp(NORM * scores).  Calibrated for the
# truncating f32->int16 convert (C=6.9).
SCH_A = float(0.25 * 128 * np.log2(np.e))
SCH_B = 16256.0 - 6.9
# per-batch exp tile indices (of 16) computed on DVE instead of ACT
DVE_EXP = frozenset({1, 4, 7, 10, 13})

_CACHE = {}
LAST_RESULT = None


def _build_nc():
    import concourse.bass as bass  # noqa: F401
    import concourse.mybir as mybir
    import concourse.tile as tile
    from concourse import bacc

    f32 = mybir.dt.float32
    f32r = mybir.dt.float32r
    bf16 = mybir.dt.bfloat16
    i16 = mybir.dt.int16
    Act = mybir.ActivationFunctionType
    Alu = mybir.AluOpType
    AX = mybir.AxisListType

    nc = bacc.Bacc("TRN2", target_bir_lowering=False, debug=False,
                   enable_asserts=False, num_devices=NCORES)

    # ---- DRAM I/O ----
    d_xW = nc.dram_tensor("xW", [3, T + E], f32r, kind="ExternalInput").ap()
    d_WqQ = nc.dram_tensor("WqQ", [E, 256], f32r, kind="ExternalInput").ap()
    d_WkQ = nc.dram_tensor("WkQ", [E, 256], f32r, kind="ExternalInput").ap()
    d_WvA = nc.dram_tensor("WvA", [E, 128], bf16, kind="ExternalInput").ap()
    d_WoA = nc.dram_tensor("WoA", [128, E], bf16, kind="ExternalInput").ap()
    d_fW1f = nc.dram_tensor("fW1f", [E, FF], f32, kind="ExternalInput").ap()
    d_fW1b = nc.dram_tensor("fW1b", [E, FF], bf16, kind="ExternalInput").ap()
    d_fW2q = nc.dram_tensor("fW2q", [128, 512], bf16, kind="ExternalInput").ap()
    d_ident = nc.dram_tensor("ident", [128, 128], bf16, kind="ExternalInput").ap()
    d_vecs = nc.dram_tensor("vecs", [128, 12], f32, kind="ExternalInput").ap()
    d_yT = nc.dram_tensor("yT", [E, T], f32, kind="ExternalOutput").ap()

    RG = [list(range(NCORES))]

    with tile.TileContext(nc) as tc:
        with tc.sbuf_pool(name="sb", bufs=1) as sb, \
             tc.psum_pool(name="ps", bufs=1) as ps, \
             tc.tile_pool(name="dr", bufs=1, space="DRAM") as dr:

            def P(shape, dt, name):  # persistent tile
                return sb.tile(shape, dt, name=name, tag=name, bufs=1)

            xW = P([3, T + E], f32r, "xW_sb")
            xTa = xW[:, :T]
            We1a_sb = xW[:, T:]
            WqQ_sb = P([128, 256], f32r, "WqQ_sb")
            WkQ_sb = P([128, 256], f32r, "WkQ_sb")
            WvA_sb = P([128, 128], bf16, "WvA_sb")
            WoA_sb = P([128, 128], bf16, "WoA_sb")
            fW1f_sb = P([128, FF], f32, "fW1f_sb")
            fW1b_sb = P([128, FF], bf16, "fW1b_sb")
            fW1s_sb = P([128, FF], f32r, "fW1s_sb")
            fW2q_sb = P([128, 512], bf16, "fW2q_sb")
            ident_sb = P([128, 128], bf16, "ident_sb")
            vecs_sb = P([128, 12], f32, "vecs_sb")

            H0T = P([128, T], f32r, "H0T")
            H0b = P([128, T], bf16, "H0b")
            qT = [P([128, T], f32r, f"qT{g}") for g in range(2)]
            kT = [P([128, T], f32r, f"kT{g}") for g in range(2)]
            V17 = P([128, 16 * 136], bf16, "V17")
            h1T = P([128, T], f32r, "h1T")
            h1nT = P([128, T], f32, "h1nT")
            yT = P([128, T], f32, "yT_sb")
            sqo = P([128, T], f32, "sqo")
            h2T = [P([128, T], bf16, f"h2T{qf}") for qf in range(4)]
            hTs = [P([128, 512], bf16, f"hTs{b}") for b in range(4)]
            sb1 = P([128, 24], f32, "sb1")
            sb2 = P([128, 24], f32, "sb2")
            mv1 = P([128, 2], f32, "mv1")
            mv2 = P([128, 2], f32, "mv2")
            G1 = P([128, 16], f32, "G1")
            G2 = P([128, 16], f32, "G2")
            cmb = P([128, 24], f32, "cmb")
            biasq = P([128, 4], f32, "biasq")
            b1pb = P([128, 1], bf16, "b1pb")

            # ---- load inputs (xTa/We1a first: they gate the embedding) ----
            nc.sync.dma_start(xW[:], d_xW)
            nc.sync.dma_start(WqQ_sb[:], d_WqQ)
            nc.sync.dma_start(WkQ_sb[:], d_WkQ)
            nc.sync.dma_start(WvA_sb[:], d_WvA)
            nc.sync.dma_start(WoA_sb[:], d_WoA)
            nc.sync.dma_start(ident_sb[:], d_ident)
            nc.sync.dma_start(vecs_sb[:], d_vecs)
            nc.sync.dma_start(fW2q_sb[:], d_fW2q)
            nc.sync.dma_start(fW1f_sb[:], d_fW1f)
            nc.sync.dma_start(fW1b_sb[:], d_fW1b)

            # ones columns of V17 (col 16 of every 17-block)
            v17v = V17.rearrange("p (t h s) -> p t h s", t=16, h=8)
            nc.gpsimd.memset(v17v[:, :, :, 16:17], 1.0)

            # pin the first ACT table load to the exp set
            nc.scalar.activation(cmb[:, 22:23], vecs_sb[:, 4:5], Act.Exp)

            # ---- Phase A: embedding  h0 = [x|1] @ [We1;be1]  (E-major) ----
            for cp in range(2):
                pm = ps.tile([128, 1024], f32, tag="sc", bufs=2, name=f"pm{cp}")
                for j in range(2):
                    c = 2 * cp + j
                    nc.tensor.matmul(pm[:, 512 * j:512 * (j + 1)],
                                     lhsT=We1a_sb,
                                     rhs=xTa[:, 512 * c:512 * (c + 1)],
                                     start=True, stop=True)
                nc.vector.tensor_copy(H0T[:, 1024 * cp:1024 * (cp + 1)], pm[:])
                nc.gpsimd.tensor_copy(H0b[:, 1024 * cp:1024 * (cp + 1)],
                                      H0T[:, 1024 * cp:1024 * (cp + 1)])

            H0r = H0T

            # ---- Phase B: q/k projections (quad-padded, f32 via DMA) ----
            for g in range(2):
                for cp in range(2):
                    pq = ps.tile([128, 1024], f32, tag="sc", bufs=2,
                                 name=f"pq{g}{cp}")
                    pk = ps.tile([128, 1024], f32, tag="sc", bufs=2,
                                 name=f"pk{g}{cp}")
                    for j in range(2):
                        c = 2 * cp + j
                        nc.tensor.matmul(pq[:, 512 * j:512 * (j + 1)],
                                         lhsT=WqQ_sb[:, 128 * g:128 * (g + 1)],
                                         rhs=H0r[:, 512 * c:512 * (c + 1)],
                                         start=True, stop=True)
                        nc.tensor.matmul(pk[:, 512 * j:512 * (j + 1)],
                                         lhsT=WkQ_sb[:, 128 * g:128 * (g + 1)],
                                         rhs=H0r[:, 512 * c:512 * (c + 1)],
                                         start=True, stop=True)
                    nc.vector.tensor_copy(qT[g][:, 1024 * cp:1024 * (cp + 1)],
                                          pq[:])
                    nc.scalar.copy(kT[g][:, 1024 * cp:1024 * (cp + 1)], pk[:])

            # ---- Phase C: v projection (emitted per-batch inside Phase D) --
            def emit_vproj(t):
                pv = ps.tile([128, 128], f32, tag="tp", bufs=1, name=f"pv{t}")
                nc.tensor.matmul(pv[:], lhsT=H0b[:, 128 * t:128 * (t + 1)],
                                 rhs=WvA_sb[:], start=True, stop=True)
                dst = v17v[:, t, :, 0:16]
                srcv = pv.rearrange("p (h s) -> p h s", h=8)
                if t % 2 == 0:
                    nc.vector.tensor_copy(dst, srcv)
                else:
                    nc.scalar.copy(dst, srcv)

            # ---- Phase D: attention, software-pipelined over batches ----
            ex_tiles = {}
            av_tiles = {}

            def score_mm(dst, b, h, c):
                g, hh = h // 4, h % 4
                nc.tensor.matmul(
                    dst,
                    lhsT=kT[g][32 * hh:32 * (hh + 1),
                               512 * b + 128 * c:512 * b + 128 * (c + 1)],
                    rhs=qT[g][32 * hh:32 * (hh + 1),
                              512 * b:512 * (b + 1)],
                    start=True, stop=True,
                    tile_position=(32 * hh, 0))

            def emit_scores_exp(b, h):
                for cp in range(2):
                    ex = sb.tile([128, 1024], bf16, tag="ex", bufs=16,
                                 name=f"ex{b}{h}{cp}")
                    if 2 * h + cp in DVE_EXP:
                        # DVE-exp tiles run on their own 1-bank rotation so
                        # they never bubble the ACT pipeline
                        for j in range(2):
                            scd = ps.tile([128, 512], f32, tag="scd", bufs=1,
                                          name=f"scd{b}{h}{cp}{j}")
                            score_mm(scd[:], b, h, 2 * cp + j)
                            nc.vector.tensor_scalar(
                                ex.bitcast(i16)[:, 512 * j:512 * (j + 1)],
                                scd[:], SCH_A, SCH_B,
                                op0=Alu.mult, op1=Alu.add)
                    else:
                        scp = ps.tile([128, 1024], f32, tag="sc", bufs=2,
                                      name=f"scp{b}{h}{cp}")
                        for j in range(2):
                            score_mm(scp[:, 512 * j:512 * (j + 1)], b, h,
                                     2 * cp + j)
                        nc.scalar.activation(ex[:], scp[:], Act.Exp,
                                             scale=float(NORM))
                    ex_tiles[(b, h, cp)] = ex

            def emit_attnv(b, h):
                # attnV: out [q, 17] per (h, qc), accumulated over 4 k-chunks
                if b not in av_tiles:
                    av_tiles[b] = ps.tile([128, 1024], f32, tag="av", bufs=1,
                                          name=f"av{b}")
                av = av_tiles[b]
                for qc in range(4):
                    for kc in range(4):
                        cp, j = kc // 2, kc % 2
                        ex = ex_tiles[(b, h, cp)]
                        nc.tensor.matmul(
                            av[:, 256 * qc + 17 * h:256 * qc + 17 * h + 17],
                            lhsT=ex[:, 512 * j + 128 * qc:
                                    512 * j + 128 * (qc + 1)],
                            rhs=V17[:, 136 * (4 * b + kc) + 17 * h:
                                    136 * (4 * b + kc) + 17 * (h + 1)],
                            start=(kc == 0), stop=(kc == 3))

            def emit_attn_tail(b):
                av = av_tiles[b]
                # normalize by the ones-column sums (per q partition)
                av4 = av.rearrange("p (q x) -> p q x", q=4)[:, :, :136]
                av5 = av4.rearrange("p q (h s) -> p q h s", h=8)
                rd = sb.tile([128, 32], f32, tag="rd", bufs=2, name=f"rd{b}")
                rd4 = rd.rearrange("p (q h s) -> p q h s", q=4, h=8)
                nc.vector.reciprocal(rd4, av5[:, :, :, 16:17])
                hn = sb.tile([128, 512], bf16, tag="hn", bufs=2, name=f"hn{b}")
                hn4 = hn.rearrange("p (q h s) -> p q h s", q=4, h=8)
                vals = av5[:, :, :, 0:16]
                rdb = bass.broadcast_tensor_aps(vals, rd4)[1]
                nc.vector.tensor_mul(hn4, vals, rdb)
                # transpose back to (h,v)-major and project
                tp = ps.tile([128, 512], bf16, tag="tp", bufs=1, name=f"tp{b}")
                for qc in range(4):
                    nc.tensor.transpose(tp[:, 128 * qc:128 * (qc + 1)],
                                        hn[:, 128 * qc:128 * (qc + 1)],
                                        ident_sb[:])
                nc.vector.tensor_copy(hTs[b][:], tp[:])
                po = ps.tile([128, 512], f32, tag="tp", bufs=1, name=f"po{b}")
                nc.tensor.matmul(po[:], lhsT=WoA_sb[:], rhs=hTs[b][:],
                                 start=True, stop=True)
                nc.vector.tensor_add(h1T[:, 512 * b:512 * (b + 1)], po[:],
                                     H0T[:, 512 * b:512 * (b + 1)])
                nc.vector.bn_stats(sb1[:, 6 * b:6 * (b + 1)],
                                   h1T[:, 512 * b:512 * (b + 1)])

            for b in range(5):
                for h in range(8):
                    if b < 4 and h < 4:
                        emit_vproj(4 * b + h)
                    if b > 0:
                        emit_attnv(b - 1, h)
                    if b < 4:
                        emit_scores_exp(b, h)
                if b > 0:
                    emit_attn_tail(b - 1)

            # ---- cross-core BN stats helper ----
            def bn_round(mv, Gt, ccname, wcol, bcol, col0):
                """AllGather per-core (mean,var); combine; produce
                a = w*rstd (cmb col0), bp = b - mean*a (col0+1)."""
                cc_in = dr.tile([128, 2], f32, name=f"{ccname}_in",
                                tag=f"{ccname}_in")
                cc_out = dr.tile([NCORES, 128, 2], f32, addr_space="Shared",
                                 name=f"{ccname}_out", tag=f"{ccname}_out")
                nc.sync.dma_start(cc_in[:], mv[:])
                nc.gpsimd.collective_compute(
                    "AllGather", Alu.bypass, replica_groups=RG,
                    ins=[cc_in[:]], outs=[cc_out[:]])
                nc.sync.dma_start(
                    Gt.rearrange("p (r s) -> p r s", r=8),
                    cc_out.rearrange("r p s -> p r s"))
                Gr = Gt.rearrange("p (r s) -> p r s", r=8)   # [128, 8, 2]
                G3 = Gr.rearrange("p r s -> p s r")          # [128, 2, 8] view
                c = cmb
                msq = c[:, col0 + 2:col0 + 10].rearrange(
                    "p (a r) -> p a r", a=1)
                nc.vector.tensor_mul(msq, G3[:, 0:1, :], G3[:, 0:1, :])
                nc.vector.tensor_add(G3[:, 1:2, :], G3[:, 1:2, :], msq)
                me2 = c[:, col0 + 4:col0 + 6]                # [mean, E2]*8
                nc.vector.reduce_sum(
                    out=me2.rearrange("p (a s) -> p a s", a=2),
                    in_=G3, axis=AX.X)
                nc.vector.tensor_scalar_mul(me2, me2, 0.125)
                mean = c[:, col0 + 4:col0 + 5]
                e2 = c[:, col0 + 5:col0 + 6]
                m2 = c[:, col0 + 6:col0 + 7]
                nc.vector.tensor_mul(m2, mean, mean)
                var = c[:, col0 + 7:col0 + 8]
                nc.vector.tensor_sub(var, e2, m2)
                sd = c[:, col0 + 8:col0 + 9]
                nc.scalar.activation(sd, var, Act.Sqrt, bias=vecs_sb[:, 4:5])
                rstd = c[:, col0 + 9:col0 + 10]
                nc.vector.reciprocal(rstd, sd)
                a = c[:, col0:col0 + 1]
                nc.vector.tensor_mul(a, rstd, vecs_sb[:, wcol:wcol + 1])
                nc.vector.tensor_mul(m2, mean, a)
                bp = c[:, col0 + 1:col0 + 2]
                nc.vector.tensor_sub(bp, vecs_sb[:, bcol:bcol + 1], m2)
                return a, bp

            # warm the sqrt table set while the collective runs (input is
            # batch-3's bn_stats count column, so this can't be hoisted into
            # the exp stream by the scheduler)
            nc.scalar.activation(cmb[:, 23:24], sb1[:, 18:19], Act.Sqrt)

            # ---- BN1 (combine + prescale W1) ----
            nc.vector.bn_aggr(mv1[:], sb1[:])
            a1, b1p = bn_round(mv1, G1, "cc1", 0, 1, 0)
            # keep the PE p-state warm through the combine so the FFN
            # matmuls start at full clock: tiny matmuls chained on the
            # successive combine outputs spread ~3us of PE activity
            for wi, wcol in enumerate((4, 5, 6, 8, 9, 0, 1)):
                wsc = ps.tile([128, 512], f32, tag="scd", bufs=1,
                              name=f"warm{wi}")
                nc.tensor.matmul(wsc[0:1, 0:1], lhsT=vecs_sb[:, 4:5],
                                 rhs=cmb[:, wcol:wcol + 1],
                                 start=True, stop=True)
            nc.vector.tensor_scalar_mul(fW1s_sb[:], fW1f_sb[:], a1)
            nc.vector.tensor_copy(b1pb[:], b1p)   # b1pb holds -n1p semantics
            pfb = ps.tile([128, 4], f32, tag="tp", bufs=1, name="pfb")
            for qf in range(4):
                nc.tensor.matmul(pfb[:, qf:qf + 1],
                                 lhsT=fW1b_sb[:, 128 * qf:128 * (qf + 1)],
                                 rhs=b1pb[:], start=True, stop=True)
            nc.vector.tensor_add(biasq[:], pfb[:], vecs_sb[:, 5:9])
            for c in range(4):
                nc.gpsimd.tensor_scalar(
                    h1nT[:, 512 * c:512 * (c + 1)],
                    h1T[:, 512 * c:512 * (c + 1)],
                    a1, b1p, op0=Alu.mult, op1=Alu.add)

            # ---- FFN (on unnormalized h1 with prescaled weights) ----
            h1r = h1T

            def ffn1(c):
                for half in range(2):
                    pf = ps.tile([128, 1024], f32, tag="sc", bufs=2,
                                 name=f"pf{c}{half}")
                    for j in range(2):
                        qf = 2 * half + j
                        nc.tensor.matmul(
                            pf[:, 512 * j:512 * (j + 1)],
                            lhsT=fW1s_sb[:, 128 * qf:128 * (qf + 1)],
                            rhs=h1r[:, 512 * c:512 * (c + 1)],
                            start=True, stop=True)
                    for j in range(2):
                        qf = 2 * half + j
                        srcr = pf[:, 512 * j:512 * (j + 1)]
                        dst = h2T[qf][:, 512 * c:512 * (c + 1)]
                        if c == 3 or qf % 2 == 0:
                            nc.scalar.activation(dst, srcr, Act.Relu,
                                                 bias=biasq[:, qf:qf + 1])
                        else:
                            nc.vector.tensor_scalar(
                                dst, srcr, biasq[:, qf:qf + 1], 0.0,
                                op0=Alu.add, op1=Alu.max)

            def ffn2(c):
                p2 = ps.tile([128, 512], f32, tag="av", bufs=1, name=f"p2{c}")
                for qf in range(4):
                    nc.tensor.matmul(p2[:],
                                     lhsT=fW2q_sb[:, 128 * qf:128 * (qf + 1)],
                                     rhs=h2T[qf][:, 512 * c:512 * (c + 1)],
                                     start=(qf == 0), stop=(qf == 3))
                nc.vector.tensor_add(yT[:, 512 * c:512 * (c + 1)], p2[:],
                                     h1nT[:, 512 * c:512 * (c + 1)])
                nc.vector.bn_stats(sb2[:, 6 * c:6 * (c + 1)],
                                   yT[:, 512 * c:512 * (c + 1)])

            ffn1(0)
            ffn1(1)
            ffn2(0)
            ffn1(2)
            ffn2(1)
            ffn1(3)
            ffn2(2)
            ffn2(3)

            # ---- BN2 + output ----
            nc.vector.bn_aggr(mv2[:], sb2[:])
            a2, b2p = bn_round(mv2, G2, "cc2", 2, 3, 12)
            for c in range(4):
                veng = nc.vector if c % 2 == 0 else nc.gpsimd
                veng.tensor_scalar(
                    sqo[:, 512 * c:512 * (c + 1)],
                    yT[:, 512 * c:512 * (c + 1)],
                    a2, b2p, op0=Alu.mult, op1=Alu.add)
                nc.sync.dma_start(d_yT[:, 512 * c:512 * (c + 1)],
                                  sqo[:, 512 * c:512 * (c + 1)])

    nc.compile()
    return nc


def _host_prep(inputs):
    f = np.float32
    Wq, Wk, Wv, Wo = (np.asarray(inputs[k], f) for k in ("Wq", "Wk", "Wv", "Wo"))
    WqQ = np.zeros((E, 256), f)
    WkQ = np.zeros((E, 256), f)
    for g in range(2):
        for hh in range(4):
            h = 4 * g + hh
            WqQ[:, 128 * g + 32 * hh:128 * g + 32 * hh + 16] = Wq[h]
            WkQ[:, 128 * g + 32 * hh:128 * g + 32 * hh + 16] = Wk[h]
    WvA = np.ascontiguousarray(np.transpose(Wv, (1, 0, 2)).reshape(E, H * KD))
    WoA = np.ascontiguousarray(Wo.reshape(H * KD, E))
    We1a = np.zeros((3, E), f)
    We1a[:2] = np.asarray(inputs["We1"], f)
    We1a[2] = np.asarray(inputs["be1"], f)
    # We1a is appended to each core's x block as cols [T:T+E] of xW
    fW1 = np.asarray(inputs["ffW1"], f)
    fW2q = np.ascontiguousarray(
        np.asarray(inputs["ffW2"], f).reshape(4, 128, E).transpose(1, 0, 2))
    vecs = np.zeros((128, 12), f)
    vecs[:, 0] = inputs["bn1_w"]
    vecs[:, 1] = inputs["bn1_b"]
    vecs[:, 2] = inputs["bn2_w"]
    vecs[:, 3] = inputs["bn2_b"]
    vecs[:, 4] = EPS
    vecs[:, 5:9] = np.asarray(inputs["ffb1"], f).reshape(4, 128).T
    import ml_dtypes
    bf = ml_dtypes.bfloat16
    return {
        "We1a_block": We1a,
        "WqQ": WqQ,
        "WkQ": WkQ,
        "WvA": WvA.astype(bf),
        "WoA": WoA.astype(bf),
        "fW1f": fW1,
        "fW1b": fW1.astype(bf),
        "fW2q": np.ascontiguousarray(fW2q.reshape(128, 512)).astype(bf),
        "ident": np.eye(128, dtype=f).astype(bf),
        "vecs": vecs,
    }


def _get_runner():
    """Build the sharded jitted executable once and cache it."""
    if "runner" in _CACHE:
        return _CACHE["runner"]
    import jax
    import concourse.mybir as mybir
    from jax.sharding import Mesh, PartitionSpec
    from jax.experimental.shard_map import shard_map
    from concourse.bass2jax import (_bass_exec_p, install_neuronx_cc_hook,
                                    partition_id_tensor)

    if "nc" not in _CACHE:
        _CACHE["nc"] = _build_nc()
    nc = _CACHE["nc"]
    install_neuronx_cc_hook()
    assert nc.dbg_addr is None

    partition_name = (nc.partition_id_tensor.name
                      if nc.partition_id_tensor else None)
    in_names, out_names, out_avals, zero_outs = [], [], [], []
    for alloc in nc.m.functions[0].allocations:
        if not isinstance(alloc, mybir.MemoryLocationSet):
            continue
        name = alloc.memorylocations[0].name
        if alloc.kind == "ExternalInput":
            if name != partition_name:
                in_names.append(name)
        elif alloc.kind == "ExternalOutput":
            shape = tuple(alloc.tensor_shape)
            dtype = mybir.dt.np(alloc.dtype)
            out_names.append(name)
            out_avals.append(jax.core.ShapedArray(shape, dtype))
            zero_outs.append(np.zeros(shape, dtype))
    n_params = len(in_names)
    n_outs = len(out_avals)
    all_in_names = list(in_names) + list(out_names)
    if partition_name is not None:
        all_in_names.append(partition_name)
    donate = tuple(range(n_params, n_params + n_outs))

    def _body(*args):
        operands = list(args)
        if partition_name is not None:
            operands.append(partition_id_tensor())
        outs = _bass_exec_p.bind(
            *operands,
            out_avals=tuple(out_avals),
            in_names=tuple(all_in_names),
            out_names=tuple(out_names),
            lowering_input_output_aliases=(),
            sim_require_finite=True,
            sim_require_nnan=True,
            nc=nc,
        )
        return tuple(outs)

    devices = jax.devices()[:NCORES]
    mesh = Mesh(np.asarray(devices), ("core",))
    in_specs = (PartitionSpec("core"),) * (n_params + n_outs)
    out_specs = (PartitionSpec("core"),) * len(out_names)
    sharded = jax.jit(
        shard_map(_body, mesh=mesh, in_specs=in_specs, out_specs=out_specs,
                  check_rep=False),
        donate_argnums=donate, keep_unused=True)

    def run(in_maps):
        per_core = [[np.asarray(m[name]) for name in in_names]
                    for m in in_maps]
        concat_in = [np.concatenate([per_core[c][i] for c in range(NCORES)],
                                    axis=0) for i in range(n_params)]
        concat_zeros = [np.zeros((NCORES * z.shape[0], *z.shape[1:]), z.dtype)
                        for z in zero_outs]
        out_arrs = sharded(*concat_in, *concat_zeros)
        out_arrs = [np.asarray(a) for a in out_arrs]
        return [{name: out_arrs[i].reshape(NCORES, *out_avals[i].shape)[c]
                 for i, name in enumerate(out_names)}
                for c in range(NCORES)]

    _CACHE["runner"] = run
    return run


def _make_in_maps(inputs):
    shared = _host_prep(inputs)
    x1 = np.asarray(inputs["x1"], np.float32)
    in_maps = []
    for cidx in range(NCORES):
        m = dict(shared)
        We1a = m.pop("We1a_block")
        xl = x1[BPC * cidx:BPC * (cidx + 1)].reshape(T, D_IN)
        xa = np.ones((3, T + E), np.float32)
        xa[:2, :T] = xl.T
        xa[:, T:] = We1a
        m["xW"] = xa
        in_maps.append(m)
    return in_maps


def kernel(**inputs):
    run = _get_runner()
    results = run(_make_in_maps(inputs))
    outs = []
    for cidx in range(NCORES):
        yTo = results[cidx]["yT"]          # [E, T]
        outs.append(np.ascontiguousarray(yTo.T).reshape(BPC, N, E))
    return np.concatenate(outs, 0).astype(np.float32)
